# revision 14
# baseline (speedup 1.0000x reference)
"""Trainium2 Bass kernel for nn_MultiHeadAttention (SL=2048, BS=2, D=1024, H=16, DH=64).

Sharding: the [BS=2, H=16] grid of attention heads is split across 8 cores:
core c handles batch b = c//4 and heads 4*(c%4) .. 4*(c%4)+4.
Each core computes q/k/v projections for its own head slice, the 4 attention
maps, and a partial output (its heads' contribution through Wo). The host
sums the 4 partials per batch.

All matmuls run as float32r (fp32 storage, FP22 multiply) at full PE rate.
Scores are computed transposed (S^T[k, q]) so softmax-exp output feeds the
AV matmul directly; softmax denominators come from an ones-matmul
(column-sum over PSUM partitions), replicated across 64 partitions so the
normalization is a plain elementwise multiply.
"""

import os
import numpy as np

SL, BS, D = 2048, 2, 1024
H, DH = 16, 64
NCORES = 8
HPC = 4            # heads per core
OD = HPC * DH      # 256 projected dims per core
DC = D // 128      # 8 contraction chunks
QC = SL // 512     # 4 query chunks of 512
KT = SL // 128     # 16 key tiles of 128

_NC = None
LAST_RESULT = None


def _build_nc():
    import concourse.mybir as mybir
    import concourse.tile as tile
    from concourse import bacc

    f32 = mybir.dt.float32
    f32r = mybir.dt.float32r
    EXP = mybir.ActivationFunctionType.Exp

    nc = bacc.Bacc(None, target_bir_lowering=False, debug=True)

    xqT = nc.dram_tensor("xqT", [D, SL], f32r, kind="ExternalInput")
    xkT = nc.dram_tensor("xkT", [D, SL], f32r, kind="ExternalInput")
    xvT = nc.dram_tensor("xvT", [D, SL], f32r, kind="ExternalInput")
    wqT = nc.dram_tensor("wqT", [D, OD], f32r, kind="ExternalInput")
    wkT = nc.dram_tensor("wkT", [D, OD], f32r, kind="ExternalInput")
    wvT = nc.dram_tensor("wvT", [D, OD], f32r, kind="ExternalInput")
    woT = nc.dram_tensor("woT", [OD, D], f32r, kind="ExternalInput")
    onesd = nc.dram_tensor("onesd", [128, 260], f32r, kind="ExternalInput")
    yT = nc.dram_tensor("yT", [D, SL], f32, kind="ExternalOutput")
    debug = bool(int(os.environ.get("KERNEL_DEBUG", "0")))
    if debug:
        dbg_qT = [nc.dram_tensor(f"dbg_qT{i}", [128, SL], f32r, kind="ExternalOutput") for i in range(2)]
        dbg_kT = [nc.dram_tensor(f"dbg_kT{i}", [128, SL], f32r, kind="ExternalOutput") for i in range(2)]
        dbg_v0 = nc.dram_tensor("dbg_v0", [128, 260], f32r, kind="ExternalOutput")
        dbg_E = nc.dram_tensor("dbg_E", [128, 1024], f32r, kind="ExternalOutput")
        dbg_OT = nc.dram_tensor("dbg_OT", [128, 512], f32r, kind="ExternalOutput")
        dbg_AVO = [nc.dram_tensor(f"dbg_AVO{i}", [65, 512], f32, kind="ExternalOutput") for i in range(2)]
        dbg_BC = [nc.dram_tensor(f"dbg_BC{i}", [64, 512], f32, kind="ExternalOutput") for i in range(2)]
        dbg_rec = [nc.dram_tensor(f"dbg_rec{i}", [1, 512], f32, kind="ExternalOutput") for i in range(2)]
        dbg_recr = [nc.dram_tensor(f"dbg_recr{i}", [1, 512], f32r, kind="ExternalOutput") for i in range(2)]

    with tile.TileContext(nc) as tc:
        with (
            tc.tile_pool(name="wsb", bufs=1) as wsb,
            tc.tile_pool(name="qk", bufs=1) as qk,
            tc.tile_pool(name="vsb", bufs=1) as vsb,
            tc.tile_pool(name="xin", bufs=16) as xin,
            tc.tile_pool(name="esb", bufs=3) as esb,
            tc.tile_pool(name="rsb", bufs=2) as rsb,
            tc.tile_pool(name="otsb", bufs=4) as otsb,
            tc.tile_pool(name="ysb", bufs=3) as ysb,
            tc.tile_pool(name="otmp", bufs=2) as otmp,
            tc.tile_pool(name="pp", bufs=1, space="PSUM") as pp,
            tc.tile_pool(name="wp", bufs=2, space="PSUM") as wp,
            tc.tile_pool(name="avop", bufs=1, space="PSUM") as avop,
            tc.tile_pool(name="yp", bufs=1, space="PSUM") as yp,
        ):
            # --- persistent SBUF tensors ---
            wq_sb = wsb.tile([128, DC * OD], f32r, tag="wq")  # [p, dc*256+od]
            wk_sb = wsb.tile([128, DC * OD], f32r, tag="wk")
            wv_sb = wsb.tile([128, DC * OD], f32r, tag="wv")
            wo_sb = wsb.tile([128, 2 * D], f32r, tag="wo")    # [p, hp*1024+o]
            ones_sb = wsb.tile([128, 260], f32r, tag="ones")
            kT_sb = [qk.tile([128, SL], f32r, tag=f"kT{ot}", name=f"kT{ot}") for ot in range(2)]
            qT_sb = [qk.tile([128, SL], f32r, tag=f"qT{ot}", name=f"qT{ot}") for ot in range(2)]
            v_sb = [vsb.tile([128, 260], f32r, tag=f"v{t}", name=f"v{t}") for t in range(KT)]

            for d in range(DC):
                nc.sync.dma_start(out=wq_sb[:, d * OD:(d + 1) * OD],
                                  in_=wqT[d * 128:(d + 1) * 128, :])
                nc.sync.dma_start(out=wk_sb[:, d * OD:(d + 1) * OD],
                                  in_=wkT[d * 128:(d + 1) * 128, :])
                nc.sync.dma_start(out=wv_sb[:, d * OD:(d + 1) * OD],
                                  in_=wvT[d * 128:(d + 1) * 128, :])
            for hp in range(2):
                nc.sync.dma_start(out=wo_sb[:, hp * D:(hp + 1) * D],
                                  in_=woT[hp * 128:(hp + 1) * 128, :])
            nc.sync.dma_start(out=ones_sb[:], in_=onesd[:])

            def load_x(xdram, cc):
                tiles = []
                for d in range(DC):
                    t = xin.tile([128, 512], f32r, tag="x", name="x")
                    nc.sync.dma_start(out=t[:],
                                      in_=xdram[d * 128:(d + 1) * 128,
                                                cc * 512:(cc + 1) * 512])
                    tiles.append(t)
                return tiles

            def proj_qk(w_sb, dst, cc, xtiles):
                # dst[ot][:, cc*512:+512] = (W x)^T for out-dims ot*128..+128
                for ot in range(2):
                    ps = pp.tile([128, 512], f32, tag="pp", name="ps")
                    for d in range(DC):
                        nc.tensor.matmul(
                            ps[:],
                            (w_sb[:, d * OD + ot * 128: d * OD + (ot + 1) * 128]),
                            (xtiles[d][:]),
                            start=(d == 0), stop=(d == DC - 1))
                    nc.vector.tensor_copy(dst[ot][:, cc * 512:(cc + 1) * 512], ps[:])

            def proj_v(cc, xtiles):
                # v_sb[t] [128 tok, 4*65]: per head 64 v-dims + a ones column
                for tt in range(4):
                    t_ = cc * 4 + tt
                    nc.sync.dma_start(out=v_sb[t_][:], in_=onesd[:])
                    ps = pp.tile([128, OD], f32, tag="pp", name="ps")
                    for d in range(DC):
                        nc.tensor.matmul(
                            ps[:],
                            (xtiles[d][:, tt * 128:(tt + 1) * 128]),
                            (wv_sb[:, d * OD:(d + 1) * OD]),
                            start=(d == 0), stop=(d == DC - 1))
                    for h in range(4):
                        nc.vector.tensor_copy(
                            v_sb[t_][:, h * 65:h * 65 + 64],
                            ps[:, h * 64:(h + 1) * 64])

            def emit_qproj(qc_):
                xt = load_x(xqT, qc_)
                proj_qk(wq_sb, qT_sb, qc_, xt)

            def emit_wo(qc_, ot_tiles, pools):
                for o8 in range(8):
                    pool, ptag = pools[o8 % len(pools)]
                    Y = pool.tile([128, 512], f32, tag=ptag, name="Y")
                    for hp in range(2):
                        nc.tensor.matmul(
                            Y[:],
                            (wo_sb[:, hp * D + o8 * 128: hp * D + (o8 + 1) * 128]),
                            (ot_tiles[hp][:]),
                            start=(hp == 0), stop=(hp == 1))
                    ys = ysb.tile([128, 512], f32, tag="ys", name="ys")
                    nc.vector.tensor_copy(ys[:], Y[:])
                    nc.sync.dma_start(
                        out=yT[o8 * 128:(o8 + 1) * 128, qc_ * 512:(qc_ + 1) * 512],
                        in_=ys[:])

            # --- projections: k (all), v (all), q (chunk 0) ---
            for cc in range(4):
                xt = load_x(xkT, cc)
                proj_qk(wk_sb, kT_sb, cc, xt)
            for cc in range(4):
                xt = load_x(xvT, cc)
                proj_v(cc, xt)
            emit_qproj(0)
            if debug:
                for i in range(2):
                    nc.sync.dma_start(out=dbg_kT[i][:], in_=kT_sb[i][:])
                nc.sync.dma_start(out=dbg_v0[:], in_=v_sb[0][:])

            # --- attention ---
            ot_prev = None
            for qc in range(QC):
                ot_cur = [None, None]
                for hp in range(2):
                    AVO = [avop.tile([65, 512], f32, tag=f"av{hip}", name="AVO")
                           for hip in range(2)]
                    for kt in range(KT):
                        W = wp.tile([128, 1024], f32, tag="wp", name="W")
                        for hip in range(2):
                            nc.tensor.matmul(
                                W[:, hip * 512:(hip + 1) * 512],
                                (kT_sb[hp][hip * 64:(hip + 1) * 64,
                                             kt * 128:(kt + 1) * 128]),
                                (qT_sb[hp][hip * 64:(hip + 1) * 64,
                                             qc * 512:(qc + 1) * 512]),
                                start=True, stop=True)
                        E = esb.tile([128, 1024], f32r, tag="E", name="E")
                        nc.scalar.activation(E[:], W[:], EXP)
                        if debug and qc == 0 and hp == 0 and kt == 0:
                            nc.sync.dma_start(out=dbg_E[:], in_=E[:])
                            for i in range(2):
                                nc.sync.dma_start(out=dbg_qT[i][:, 0:512],
                                                  in_=qT_sb[i][:, 0:512])
                        for hip in range(2):
                            nc.tensor.matmul(
                                AVO[hip][:],
                                (v_sb[kt][:, (hp * 2 + hip) * 65:
                                            (hp * 2 + hip) * 65 + 65]),
                                (E[:, hip * 512:(hip + 1) * 512]),
                                start=(kt == 0), stop=(kt == KT - 1))
                        if hp == 0 and kt == 2 and qc < QC - 1:
                            emit_qproj(qc + 1)
                        if hp == 1 and kt == 4 and ot_prev is not None:
                            emit_wo(qc - 1, ot_prev, [(yp, 'yp')])
                            ot_prev = None
                    OT = otsb.tile([128, 512], f32r, tag="ot", name="OT")
                    for hip in range(2):
                        sums_r = rsb.tile([65, 512], f32r, tag="recip", name="sums_r")
                        nc.vector.tensor_copy(sums_r[64:65, :], AVO[hip][64:65, :])
                        BCp = pp.tile([64, 512], f32, tag="pp", name="BCp")
                        nc.tensor.matmul(BCp[:], ones_sb[64:65, 0:64],
                                         sums_r[64:65, :],
                                         start=True, stop=True)
                        sumsb = ysb.tile([64, 512], f32, tag="ys", name="sumsb")
                        nc.vector.tensor_copy(sumsb[:], BCp[:])
                        BCs = ysb.tile([64, 512], f32, tag="ys", name="BCs")
                        nc.vector.reciprocal_approx_fast(BCs[:], sumsb[:])
                        if debug and qc == 0 and hp == 0:
                            avod = ysb.tile([65, 512], f32, tag="ys", name="avod")
                            nc.vector.tensor_copy(avod[:], AVO[hip][:])
                            nc.sync.dma_start(out=dbg_AVO[hip][:], in_=avod[:])
                            nc.sync.dma_start(out=dbg_BC[hip][:], in_=BCs[:])
                            nc.sync.dma_start(out=dbg_rec[hip][:], in_=BCs[0:1, :])
                            nc.sync.dma_start(out=dbg_recr[hip][:], in_=sums_r[64:65, :])
                        if hip == 0:
                            nc.vector.tensor_mul(OT[0:64, :], AVO[0][0:64, :], BCs[:])
                        else:
                            OTt = otmp.tile([64, 512], f32r, tag="otmp", name="OTt")
                            nc.vector.tensor_mul(OTt[:], AVO[1][0:64, :], BCs[:])
                            nc.sync.dma_start(out=OT[64:128, :], in_=OTt[:])
                    if debug and qc == 0 and hp == 0:
                        nc.sync.dma_start(out=dbg_OT[:], in_=OT[:])
                    ot_cur[hp] = OT
                ot_prev = ot_cur

            emit_wo(QC - 1, ot_prev, [(yp, 'yp'), (avop, 'av0'), (avop, 'av1')])

    nc.compile()
    return nc


def _get_nc():
    global _NC
    if _NC is None:
        _NC = _build_nc()
    return _NC


def _host_fallback(query, keys, values, mask, Wq, Wk, Wv, Wo):
    # Exact reference math in numpy; only used if mask has zeros (off-spec).
    q = (query @ Wq.T).reshape(SL, BS, H, DH)
    k = (keys @ Wk.T).reshape(SL, BS, H, DH)
    v = (values @ Wv.T).reshape(SL, BS, H, DH)
    out = np.zeros((SL, BS, H * DH), np.float32)
    for b in range(BS):
        for h in range(H):
            s = q[:, b, h, :] @ k[:, b, h, :].T
            s = np.where(mask[0, 0] == 0, np.float32(-1e20), s)
            s = s - s.max(axis=-1, keepdims=True)
            p = np.exp(s)
            p /= p.sum(axis=-1, keepdims=True)
            out[:, b, h * DH:(h + 1) * DH] = p @ v[:, b, h, :]
    return out @ Wo.T


def _enable_trace_support():
    """Install the antenv.axon_hooks shim so trace=True works under axon."""
    import sys
    import types
    import antenv
    if "antenv.axon_hooks" in sys.modules:
        return
    hookmod = types.ModuleType("antenv.axon_hooks")
    _hook = [None]
    hookmod.set_axon_ntff_profile_hook = lambda h: _hook.__setitem__(0, h)
    hookmod.get_axon_ntff_profile_hook = lambda: _hook[0]
    antenv.axon_hooks = hookmod
    sys.modules["antenv.axon_hooks"] = hookmod
    try:
        from trn_agent_boot.trn_boot import _ntff_profile_via_ctypes
        hookmod.set_axon_ntff_profile_hook(
            _ntff_profile_via_ctypes("/opt/axon/libaxon_pjrt.so"))
    except Exception:
        pass
    import concourse.bass_utils as bu
    bu.upload_artifacts = lambda tmpdir: tmpdir


def kernel(query, keys, values, mask, Wq, Wk, Wv, Wo):
    query = np.asarray(query, np.float32)
    keys = np.asarray(keys, np.float32)
    values = np.asarray(values, np.float32)
    mask = np.asarray(mask)
    Wq = np.asarray(Wq, np.float32)
    Wk = np.asarray(Wk, np.float32)
    Wv = np.asarray(Wv, np.float32)
    Wo = np.asarray(Wo, np.float32)

    if (mask == 0).any():
        return _host_fallback(query, keys, values, mask, Wq, Wk, Wv, Wo)

    trace = bool(int(os.environ.get("KERNEL_TRACE", "0")))
    if trace:
        _enable_trace_support()

    from concourse.bass_utils import run_bass_kernel_spmd

    nc = _get_nc()
    in_maps = []
    for c in range(NCORES):
        b, hg = divmod(c, 4)
        hs = hg * OD
        in_maps.append({
            "xqT": np.ascontiguousarray(query[:, b, :].T),
            "xkT": np.ascontiguousarray(keys[:, b, :].T),
            "xvT": np.ascontiguousarray(values[:, b, :].T),
            "wqT": np.ascontiguousarray(Wq[hs:hs + OD, :].T),
            "wkT": np.ascontiguousarray(Wk[hs:hs + OD, :].T),
            "wvT": np.ascontiguousarray(Wv[hs:hs + OD, :].T),
            "woT": np.ascontiguousarray(Wo[:, hs:hs + OD].T),
            "onesd": np.ones((128, 260), np.float32),
        })

    res = run_bass_kernel_spmd(nc, in_maps, core_ids=list(range(NCORES)),
                               trace=trace)
    global LAST_RESULT
    LAST_RESULT = res

    out = np.zeros((SL, BS, D), np.float32)
    for c in range(NCORES):
        b = c // 4
        out[:, b, :] += res.results[c]["yT"].T
    return out


# revision 15
# speedup vs baseline: 1.0832x; 1.0832x over previous
"""Trainium2 Bass kernel for nn_MultiHeadAttention (SL=2048, BS=2, D=1024, H=16, DH=64).

Sharding: the [BS=2, H=16] grid of attention heads is split across 8 cores:
core c handles batch b = c//4 and heads 4*(c%4) .. 4*(c%4)+4.
Each core computes q/k/v projections for its own head slice, the 4 attention
maps, and a partial output (its heads' contribution through Wo). The host
sums the 4 partials per batch.

All matmuls run as float32r (fp32 storage, FP22 multiply) at full PE rate.
Scores are computed transposed (S^T[k, q]) so softmax-exp output feeds the
AV matmul directly; softmax denominators come from an ones-matmul
(column-sum over PSUM partitions), replicated across 64 partitions so the
normalization is a plain elementwise multiply.
"""

import os
import numpy as np

SL, BS, D = 2048, 2, 1024
H, DH = 16, 64
NCORES = 8
HPC = 4            # heads per core
OD = HPC * DH      # 256 projected dims per core
DC = D // 128      # 8 contraction chunks
QC = SL // 512     # 4 query chunks of 512
KT = SL // 128     # 16 key tiles of 128

_NC = None
LAST_RESULT = None


def _build_nc():
    import concourse.mybir as mybir
    import concourse.tile as tile
    from concourse import bacc

    f32 = mybir.dt.float32
    f32r = mybir.dt.float32r
    EXP = mybir.ActivationFunctionType.Exp

    nc = bacc.Bacc(None, target_bir_lowering=False, debug=True)

    xqT = nc.dram_tensor("xqT", [D, SL], f32r, kind="ExternalInput")
    xkT = nc.dram_tensor("xkT", [D, SL], f32r, kind="ExternalInput")
    xvT = nc.dram_tensor("xvT", [D, SL], f32r, kind="ExternalInput")
    wqT = nc.dram_tensor("wqT", [D, OD], f32r, kind="ExternalInput")
    wkT = nc.dram_tensor("wkT", [D, OD], f32r, kind="ExternalInput")
    wvT = nc.dram_tensor("wvT", [D, OD], f32r, kind="ExternalInput")
    woT = nc.dram_tensor("woT", [OD, D], f32r, kind="ExternalInput")
    onesd = nc.dram_tensor("onesd", [128, 260], f32r, kind="ExternalInput")
    yT = nc.dram_tensor("yT", [D, SL], f32, kind="ExternalOutput")
    debug = bool(int(os.environ.get("KERNEL_DEBUG", "0")))
    if debug:
        dbg_qT = [nc.dram_tensor(f"dbg_qT{i}", [128, SL], f32r, kind="ExternalOutput") for i in range(2)]
        dbg_kT = [nc.dram_tensor(f"dbg_kT{i}", [128, SL], f32r, kind="ExternalOutput") for i in range(2)]
        dbg_v0 = nc.dram_tensor("dbg_v0", [128, 260], f32r, kind="ExternalOutput")
        dbg_E = nc.dram_tensor("dbg_E", [128, 1024], f32r, kind="ExternalOutput")
        dbg_OT = nc.dram_tensor("dbg_OT", [128, 512], f32r, kind="ExternalOutput")
        dbg_AVO = [nc.dram_tensor(f"dbg_AVO{i}", [65, 512], f32, kind="ExternalOutput") for i in range(2)]
        dbg_BC = [nc.dram_tensor(f"dbg_BC{i}", [64, 512], f32, kind="ExternalOutput") for i in range(2)]
        dbg_rec = [nc.dram_tensor(f"dbg_rec{i}", [1, 512], f32, kind="ExternalOutput") for i in range(2)]
        dbg_recr = [nc.dram_tensor(f"dbg_recr{i}", [1, 512], f32r, kind="ExternalOutput") for i in range(2)]

    with tile.TileContext(nc) as tc:
        with (
            tc.tile_pool(name="wsb", bufs=1) as wsb,
            tc.tile_pool(name="qk", bufs=1) as qk,
            tc.tile_pool(name="vsb", bufs=1) as vsb,
            tc.tile_pool(name="xin", bufs=16) as xin,
            tc.tile_pool(name="esb", bufs=3) as esb,
            tc.tile_pool(name="rsb", bufs=2) as rsb,
            tc.tile_pool(name="otsb", bufs=4) as otsb,
            tc.tile_pool(name="ysb", bufs=3) as ysb,
            tc.tile_pool(name="otmp", bufs=2) as otmp,
            tc.tile_pool(name="avsb", bufs=2) as avsb,
            tc.tile_pool(name="pp", bufs=1, space="PSUM") as pp,
            tc.tile_pool(name="wp", bufs=2, space="PSUM") as wp,
            tc.tile_pool(name="avop", bufs=1, space="PSUM") as avop,
            tc.tile_pool(name="yp", bufs=1, space="PSUM") as yp,
        ):
            # --- persistent SBUF tensors ---
            wq_sb = wsb.tile([128, DC * OD], f32r, tag="wq")  # [p, dc*256+od]
            wk_sb = wsb.tile([128, DC * OD], f32r, tag="wk")
            wv_sb = wsb.tile([128, DC * OD], f32r, tag="wv")
            wo_sb = wsb.tile([128, 2 * D], f32r, tag="wo")    # [p, hp*1024+o]
            ones_sb = wsb.tile([128, 260], f32r, tag="ones")
            kT_sb = [qk.tile([128, SL], f32r, tag=f"kT{ot}", name=f"kT{ot}") for ot in range(2)]
            qT_sb = [qk.tile([128, SL], f32r, tag=f"qT{ot}", name=f"qT{ot}") for ot in range(2)]
            v_sb = [vsb.tile([128, 260], f32r, tag=f"v{t}", name=f"v{t}") for t in range(KT)]

            def load_w(dst, src):
                for d in range(DC):
                    nc.sync.dma_start(out=dst[:, d * OD:(d + 1) * OD],
                                      in_=src[d * 128:(d + 1) * 128, :])

            def load_x(xdram, cc):
                tiles = []
                for d in range(DC):
                    t = xin.tile([128, 512], f32r, tag="x", name="x")
                    nc.sync.dma_start(out=t[:],
                                      in_=xdram[d * 128:(d + 1) * 128,
                                                cc * 512:(cc + 1) * 512])
                    tiles.append(t)
                return tiles

            def proj_qk(w_sb, dst, cc, xtiles):
                # dst[ot][:, cc*512:+512] = (W x)^T for out-dims ot*128..+128
                for ot in range(2):
                    ps = pp.tile([128, 512], f32, tag="pp", name="ps")
                    for d in range(DC):
                        nc.tensor.matmul(
                            ps[:],
                            (w_sb[:, d * OD + ot * 128: d * OD + (ot + 1) * 128]),
                            (xtiles[d][:]),
                            start=(d == 0), stop=(d == DC - 1))
                    nc.vector.tensor_copy(dst[ot][:, cc * 512:(cc + 1) * 512], ps[:])

            def proj_v(cc, xtiles):
                # v_sb[t] [128 tok, 4*65]: per head 64 v-dims + a ones column
                for tt in range(4):
                    t_ = cc * 4 + tt
                    nc.sync.dma_start(out=v_sb[t_][:], in_=onesd[:])
                    ps = pp.tile([128, OD], f32, tag="pp", name="ps")
                    for d in range(DC):
                        nc.tensor.matmul(
                            ps[:],
                            (xtiles[d][:, tt * 128:(tt + 1) * 128]),
                            (wv_sb[:, d * OD:(d + 1) * OD]),
                            start=(d == 0), stop=(d == DC - 1))
                    for h in range(4):
                        nc.vector.tensor_copy(
                            v_sb[t_][:, h * 65:h * 65 + 64],
                            ps[:, h * 64:(h + 1) * 64])

            def emit_qproj(qc_):
                xt = load_x(xqT, qc_)
                proj_qk(wq_sb, qT_sb, qc_, xt)

            def emit_wo(qc_, ot_tiles, pools):
                for o8 in range(8):
                    pool, ptag = pools[o8 % len(pools)]
                    Y = pool.tile([128, 512], f32, tag=ptag, name="Y")
                    for hp in range(2):
                        nc.tensor.matmul(
                            Y[:],
                            (wo_sb[:, hp * D + o8 * 128: hp * D + (o8 + 1) * 128]),
                            (ot_tiles[hp][:]),
                            start=(hp == 0), stop=(hp == 1))
                    ys = ysb.tile([128, 512], f32, tag="ys", name="ys")
                    nc.vector.tensor_copy(ys[:], Y[:])
                    nc.sync.dma_start(
                        out=yT[o8 * 128:(o8 + 1) * 128, qc_ * 512:(qc_ + 1) * 512],
                        in_=ys[:])

            # --- projections: k (all), v (all), q (chunk 0) ---
            load_w(wk_sb, wkT)
            nc.sync.dma_start(out=ones_sb[:], in_=onesd[:])
            for cc in range(4):
                xt = load_x(xkT, cc)
                if cc == 0:
                    load_w(wv_sb, wvT)
                proj_qk(wk_sb, kT_sb, cc, xt)
            for cc in range(4):
                xt = load_x(xvT, cc)
                if cc == 0:
                    load_w(wq_sb, wqT)
                proj_v(cc, xt)
            emit_qproj(0)
            for hp in range(2):
                nc.sync.dma_start(out=wo_sb[:, hp * D:(hp + 1) * D],
                                  in_=woT[hp * 128:(hp + 1) * 128, :])
            if debug:
                for i in range(2):
                    nc.sync.dma_start(out=dbg_kT[i][:], in_=kT_sb[i][:])
                nc.sync.dma_start(out=dbg_v0[:], in_=v_sb[0][:])

            # --- attention ---
            ot_prev = None
            for qc in range(QC):
                ot_cur = [None, None]
                for hp in range(2):
                    AVO = [avop.tile([65, 512], f32, tag=f"av{hip}", name="AVO")
                           for hip in range(2)]

                    def emit_av(E_, kt_):
                        for hip in range(2):
                            nc.tensor.matmul(
                                AVO[hip][:],
                                (v_sb[kt_][:, (hp * 2 + hip) * 65:
                                             (hp * 2 + hip) * 65 + 65]),
                                (E_[:, hip * 512:(hip + 1) * 512]),
                                start=(kt_ == 0), stop=(kt_ == KT - 1))

                    prev_E = None
                    for kt in range(KT):
                        W = wp.tile([128, 1024], f32, tag="wp", name="W")
                        for hip in range(2):
                            nc.tensor.matmul(
                                W[:, hip * 512:(hip + 1) * 512],
                                (kT_sb[hp][hip * 64:(hip + 1) * 64,
                                             kt * 128:(kt + 1) * 128]),
                                (qT_sb[hp][hip * 64:(hip + 1) * 64,
                                             qc * 512:(qc + 1) * 512]),
                                start=True, stop=True)
                        E = esb.tile([128, 1024], f32r, tag="E", name="E")
                        nc.scalar.activation(E[:], W[:], EXP)
                        if debug and qc == 0 and hp == 0 and kt == 0:
                            nc.sync.dma_start(out=dbg_E[:], in_=E[:])
                            for i in range(2):
                                nc.sync.dma_start(out=dbg_qT[i][:, 0:512],
                                                  in_=qT_sb[i][:, 0:512])
                        if prev_E is not None:
                            emit_av(prev_E, kt - 1)
                        prev_E = E
                        if hp == 0 and kt == 2 and qc < QC - 1:
                            emit_qproj(qc + 1)
                        if hp == 1 and kt == 4 and ot_prev is not None:
                            emit_wo(qc - 1, ot_prev, [(yp, 'yp')])
                            ot_prev = None
                    emit_av(prev_E, KT - 1)
                    OT = otsb.tile([128, 512], f32r, tag="ot", name="OT")
                    for hip in range(2):
                        avs = avsb.tile([65, 512], f32, tag="avs", name="avs")
                        nc.vector.tensor_copy(avs[:], AVO[hip][:])
                        sums_r = rsb.tile([65, 512], f32r, tag="recip", name="sums_r")
                        nc.vector.tensor_copy(sums_r[64:65, :], avs[64:65, :])
                        BCp = pp.tile([64, 512], f32, tag="pp", name="BCp")
                        nc.tensor.matmul(BCp[:], ones_sb[64:65, 0:64],
                                         sums_r[64:65, :],
                                         start=True, stop=True)
                        sumsb = ysb.tile([64, 512], f32, tag="ys", name="sumsb")
                        nc.vector.tensor_copy(sumsb[:], BCp[:])
                        BCs = ysb.tile([64, 512], f32, tag="ys", name="BCs")
                        nc.vector.reciprocal_approx_fast(BCs[:], sumsb[:])
                        if debug and qc == 0 and hp == 0:
                            nc.sync.dma_start(out=dbg_AVO[hip][:], in_=avs[:])
                            nc.sync.dma_start(out=dbg_BC[hip][:], in_=BCs[:])
                            nc.sync.dma_start(out=dbg_rec[hip][:], in_=BCs[0:1, :])
                            nc.sync.dma_start(out=dbg_recr[hip][:], in_=sums_r[64:65, :])
                        if hip == 0:
                            nc.vector.tensor_mul(OT[0:64, :], avs[0:64, :], BCs[:])
                        else:
                            OTt = otmp.tile([64, 512], f32r, tag="otmp", name="OTt")
                            nc.vector.tensor_mul(OTt[:], avs[0:64, :], BCs[:])
                            nc.sync.dma_start(out=OT[64:128, :], in_=OTt[:])
                    if debug and qc == 0 and hp == 0:
                        nc.sync.dma_start(out=dbg_OT[:], in_=OT[:])
                    ot_cur[hp] = OT
                ot_prev = ot_cur

            emit_wo(QC - 1, ot_prev, [(yp, 'yp'), (avop, 'av0'), (avop, 'av1')])

    nc.compile()
    return nc


def _get_nc():
    global _NC
    if _NC is None:
        _NC = _build_nc()
    return _NC


def _host_fallback(query, keys, values, mask, Wq, Wk, Wv, Wo):
    # Exact reference math in numpy; only used if mask has zeros (off-spec).
    q = (query @ Wq.T).reshape(SL, BS, H, DH)
    k = (keys @ Wk.T).reshape(SL, BS, H, DH)
    v = (values @ Wv.T).reshape(SL, BS, H, DH)
    out = np.zeros((SL, BS, H * DH), np.float32)
    for b in range(BS):
        for h in range(H):
            s = q[:, b, h, :] @ k[:, b, h, :].T
            s = np.where(mask[0, 0] == 0, np.float32(-1e20), s)
            s = s - s.max(axis=-1, keepdims=True)
            p = np.exp(s)
            p /= p.sum(axis=-1, keepdims=True)
            out[:, b, h * DH:(h + 1) * DH] = p @ v[:, b, h, :]
    return out @ Wo.T


def _enable_trace_support():
    """Install the antenv.axon_hooks shim so trace=True works under axon."""
    import sys
    import types
    import antenv
    if "antenv.axon_hooks" in sys.modules:
        return
    hookmod = types.ModuleType("antenv.axon_hooks")
    _hook = [None]
    hookmod.set_axon_ntff_profile_hook = lambda h: _hook.__setitem__(0, h)
    hookmod.get_axon_ntff_profile_hook = lambda: _hook[0]
    antenv.axon_hooks = hookmod
    sys.modules["antenv.axon_hooks"] = hookmod
    try:
        from trn_agent_boot.trn_boot import _ntff_profile_via_ctypes
        hookmod.set_axon_ntff_profile_hook(
            _ntff_profile_via_ctypes("/opt/axon/libaxon_pjrt.so"))
    except Exception:
        pass
    import concourse.bass_utils as bu
    bu.upload_artifacts = lambda tmpdir: tmpdir


def kernel(query, keys, values, mask, Wq, Wk, Wv, Wo):
    query = np.asarray(query, np.float32)
    keys = np.asarray(keys, np.float32)
    values = np.asarray(values, np.float32)
    mask = np.asarray(mask)
    Wq = np.asarray(Wq, np.float32)
    Wk = np.asarray(Wk, np.float32)
    Wv = np.asarray(Wv, np.float32)
    Wo = np.asarray(Wo, np.float32)

    if (mask == 0).any():
        return _host_fallback(query, keys, values, mask, Wq, Wk, Wv, Wo)

    trace = bool(int(os.environ.get("KERNEL_TRACE", "0")))
    if trace:
        _enable_trace_support()

    from concourse.bass_utils import run_bass_kernel_spmd

    nc = _get_nc()
    in_maps = []
    for c in range(NCORES):
        b, hg = divmod(c, 4)
        hs = hg * OD
        in_maps.append({
            "xqT": np.ascontiguousarray(query[:, b, :].T),
            "xkT": np.ascontiguousarray(keys[:, b, :].T),
            "xvT": np.ascontiguousarray(values[:, b, :].T),
            "wqT": np.ascontiguousarray(Wq[hs:hs + OD, :].T),
            "wkT": np.ascontiguousarray(Wk[hs:hs + OD, :].T),
            "wvT": np.ascontiguousarray(Wv[hs:hs + OD, :].T),
            "woT": np.ascontiguousarray(Wo[:, hs:hs + OD].T),
            "onesd": np.ones((128, 260), np.float32),
        })

    res = run_bass_kernel_spmd(nc, in_maps, core_ids=list(range(NCORES)),
                               trace=trace)
    global LAST_RESULT
    LAST_RESULT = res

    out = np.zeros((SL, BS, D), np.float32)
    for c in range(NCORES):
        b = c // 4
        out[:, b, :] += res.results[c]["yT"].T
    return out


# revision 16
# speedup vs baseline: 1.1959x; 1.1040x over previous
"""Trainium2 Bass kernel for nn_MultiHeadAttention (SL=2048, BS=2, D=1024, H=16, DH=64).

Sharding: the [BS=2, H=16] grid of attention heads is split across 8 cores:
core c handles batch b = c//4 and heads 4*(c%4) .. 4*(c%4)+4.
Each core computes q/k/v projections for its own head slice, the 4 attention
maps, and a partial output (its heads' contribution through Wo). The host
sums the 4 partials per batch.

All matmuls run as float32r (fp32 storage, FP22 multiply) at full PE rate.
Scores are computed transposed (S^T[k, q]) so softmax-exp output feeds the
AV matmul directly; softmax denominators come from an ones-matmul
(column-sum over PSUM partitions), replicated across 64 partitions so the
normalization is a plain elementwise multiply.
"""

import os
import numpy as np

SL, BS, D = 2048, 2, 1024
H, DH = 16, 64
NCORES = 8
HPC = 4            # heads per core
OD = HPC * DH      # 256 projected dims per core
DC = D // 128      # 8 contraction chunks
QC = SL // 512     # 4 query chunks of 512
KT = SL // 128     # 16 key tiles of 128

_NC = None
LAST_RESULT = None


def _build_nc():
    import concourse.mybir as mybir
    import concourse.tile as tile
    from concourse import bacc

    f32 = mybir.dt.float32
    f32r = mybir.dt.float32r
    EXP = mybir.ActivationFunctionType.Exp

    nc = bacc.Bacc(None, target_bir_lowering=False, debug=True)

    xqT = nc.dram_tensor("xqT", [D, SL], f32r, kind="ExternalInput")
    xkT = nc.dram_tensor("xkT", [D, SL], f32r, kind="ExternalInput")
    xvT = nc.dram_tensor("xvT", [D, SL], f32r, kind="ExternalInput")
    wqT = nc.dram_tensor("wqT", [D, OD], f32r, kind="ExternalInput")
    wkT = nc.dram_tensor("wkT", [D, OD], f32r, kind="ExternalInput")
    wvT = nc.dram_tensor("wvT", [D, OD], f32r, kind="ExternalInput")
    woT = nc.dram_tensor("woT", [OD, D], f32r, kind="ExternalInput")
    onesd = nc.dram_tensor("onesd", [128, 260], f32r, kind="ExternalInput")
    yT = nc.dram_tensor("yT", [D, SL], f32, kind="ExternalOutput")
    debug = bool(int(os.environ.get("KERNEL_DEBUG", "0")))
    if debug:
        dbg_qT = [nc.dram_tensor(f"dbg_qT{i}", [128, SL], f32r, kind="ExternalOutput") for i in range(2)]
        dbg_kT = [nc.dram_tensor(f"dbg_kT{i}", [128, SL], f32r, kind="ExternalOutput") for i in range(2)]
        dbg_v0 = nc.dram_tensor("dbg_v0", [128, 260], f32r, kind="ExternalOutput")
        dbg_E = nc.dram_tensor("dbg_E", [128, 1024], f32r, kind="ExternalOutput")
        dbg_OT = nc.dram_tensor("dbg_OT", [128, 512], f32r, kind="ExternalOutput")
        dbg_AVO = [nc.dram_tensor(f"dbg_AVO{i}", [65, 512], f32, kind="ExternalOutput") for i in range(2)]
        dbg_BC = [nc.dram_tensor(f"dbg_BC{i}", [64, 512], f32, kind="ExternalOutput") for i in range(2)]
        dbg_rec = [nc.dram_tensor(f"dbg_rec{i}", [1, 512], f32, kind="ExternalOutput") for i in range(2)]
        dbg_recr = [nc.dram_tensor(f"dbg_recr{i}", [1, 512], f32r, kind="ExternalOutput") for i in range(2)]

    with tile.TileContext(nc) as tc:
        with (
            tc.tile_pool(name="wsb", bufs=1) as wsb,
            tc.tile_pool(name="qk", bufs=1) as qk,
            tc.tile_pool(name="vsb", bufs=1) as vsb,
            tc.tile_pool(name="xin", bufs=24) as xin,
            tc.tile_pool(name="esb", bufs=3) as esb,
            tc.tile_pool(name="rsb", bufs=2) as rsb,
            tc.tile_pool(name="otsb", bufs=4) as otsb,
            tc.tile_pool(name="ysb", bufs=3) as ysb,
            tc.tile_pool(name="otmp", bufs=2) as otmp,
            tc.tile_pool(name="avsb", bufs=2) as avsb,
            tc.tile_pool(name="pp", bufs=1, space="PSUM") as pp,
            tc.tile_pool(name="wp", bufs=2, space="PSUM") as wp,
            tc.tile_pool(name="avop", bufs=1, space="PSUM") as avop,
            tc.tile_pool(name="yp", bufs=1, space="PSUM") as yp,
        ):
            # --- persistent SBUF tensors ---
            wq_sb = wsb.tile([128, DC * OD], f32r, tag="wq")  # [p, dc*256+od]
            wk_sb = wsb.tile([128, DC * OD], f32r, tag="wk")
            wv_sb = wsb.tile([128, DC * OD], f32r, tag="wv")
            wo_sb = wsb.tile([128, 2 * D], f32r, tag="wo")    # [p, hp*1024+o]
            ones_sb = wsb.tile([128, 260], f32r, tag="ones")
            kT_sb = [qk.tile([128, SL], f32r, tag=f"kT{ot}", name=f"kT{ot}") for ot in range(2)]
            qT_sb = [qk.tile([128, SL], f32r, tag=f"qT{ot}", name=f"qT{ot}") for ot in range(2)]
            v_sb = [vsb.tile([128, 260], f32r, tag=f"v{t}", name=f"v{t}") for t in range(KT)]

            def load_w(dst, src):
                for d in range(DC):
                    nc.sync.dma_start(out=dst[:, d * OD:(d + 1) * OD],
                                      in_=src[d * 128:(d + 1) * 128, :])

            def load_x(xdram, cc):
                tiles = []
                for d in range(DC):
                    t = xin.tile([128, 512], f32r, tag="x", name="x")
                    nc.sync.dma_start(out=t[:],
                                      in_=xdram[d * 128:(d + 1) * 128,
                                                cc * 512:(cc + 1) * 512])
                    tiles.append(t)
                return tiles

            def proj_qk(w_sb, dst, cc, xtiles):
                # dst[ot][:, cc*512:+512] = (W x)^T for out-dims ot*128..+128
                for ot in range(2):
                    ps = pp.tile([128, 512], f32, tag="pp", name="ps")
                    for d in range(DC):
                        nc.tensor.matmul(
                            ps[:],
                            (w_sb[:, d * OD + ot * 128: d * OD + (ot + 1) * 128]),
                            (xtiles[d][:]),
                            start=(d == 0), stop=(d == DC - 1))
                    nc.vector.tensor_copy(dst[ot][:, cc * 512:(cc + 1) * 512], ps[:])

            def emit_qproj(qc_):
                xt = load_x(xqT, qc_)
                proj_qk(wq_sb, qT_sb, qc_, xt)

            def emit_wo(qc_, ot_tiles, pools):
                for o8 in range(8):
                    pool, ptag = pools[o8 % len(pools)]
                    Y = pool.tile([128, 512], f32, tag=ptag, name="Y")
                    for hp in range(2):
                        nc.tensor.matmul(
                            Y[:],
                            (wo_sb[:, hp * D + o8 * 128: hp * D + (o8 + 1) * 128]),
                            (ot_tiles[hp][:]),
                            start=(hp == 0), stop=(hp == 1))
                    ys = ysb.tile([128, 512], f32, tag="ys", name="ys")
                    nc.vector.tensor_copy(ys[:], Y[:])
                    nc.sync.dma_start(
                        out=yT[o8 * 128:(o8 + 1) * 128, qc_ * 512:(qc_ + 1) * 512],
                        in_=ys[:])

            # --- projections: k fully, q chunk 0; v is produced inside the
            # first attention ladder, one 128-token tile per window ---
            load_w(wk_sb, wkT)
            nc.sync.dma_start(out=ones_sb[:], in_=onesd[:])
            xq0 = load_x(xqT, 0)
            load_w(wq_sb, wqT)
            for cc in range(4):
                xt = load_x(xkT, cc)
                if cc == 0:
                    load_w(wv_sb, wvT)
                proj_qk(wk_sb, kT_sb, cc, xt)
            proj_qk(wq_sb, qT_sb, 0, xq0)
            for hp in range(2):
                nc.sync.dma_start(out=wo_sb[:, hp * D:(hp + 1) * D],
                                  in_=woT[hp * 128:(hp + 1) * 128, :])
            xv_chunks = {0: load_x(xvT, 0)}

            def emit_vtile(t_):
                cc_, tt = divmod(t_, 4)
                xtiles = xv_chunks[cc_]
                nc.sync.dma_start(out=v_sb[t_][:], in_=onesd[:])
                ps = pp.tile([128, OD], f32, tag="pp", name="ps")
                for d in range(DC):
                    nc.tensor.matmul(
                        ps[:],
                        (xtiles[d][:, tt * 128:(tt + 1) * 128]),
                        (wv_sb[:, d * OD:(d + 1) * OD]),
                        start=(d == 0), stop=(d == DC - 1))
                for h in range(4):
                    nc.vector.tensor_copy(
                        v_sb[t_][:, h * 65:h * 65 + 64],
                        ps[:, h * 64:(h + 1) * 64])
            if debug:
                for i in range(2):
                    nc.sync.dma_start(out=dbg_kT[i][:], in_=kT_sb[i][:])
                nc.sync.dma_start(out=dbg_v0[:], in_=v_sb[0][:])

            # --- attention ---
            ot_prev = None
            for qc in range(QC):
                ot_cur = [None, None]
                for hp in range(2):
                    AVO = [avop.tile([65, 512], f32, tag=f"av{hip}", name="AVO")
                           for hip in range(2)]

                    def emit_av(E_, kt_):
                        for hip in range(2):
                            nc.tensor.matmul(
                                AVO[hip][:],
                                (v_sb[kt_][:, (hp * 2 + hip) * 65:
                                             (hp * 2 + hip) * 65 + 65]),
                                (E_[:, hip * 512:(hip + 1) * 512]),
                                start=(kt_ == 0), stop=(kt_ == KT - 1))

                    prev_E = None
                    for kt in range(KT):
                        W = wp.tile([128, 1024], f32, tag="wp", name="W")
                        for hip in range(2):
                            nc.tensor.matmul(
                                W[:, hip * 512:(hip + 1) * 512],
                                (kT_sb[hp][hip * 64:(hip + 1) * 64,
                                             kt * 128:(kt + 1) * 128]),
                                (qT_sb[hp][hip * 64:(hip + 1) * 64,
                                             qc * 512:(qc + 1) * 512]),
                                start=True, stop=True)
                        E = esb.tile([128, 1024], f32r, tag="E", name="E")
                        nc.scalar.activation(E[:], W[:], EXP)
                        if debug and qc == 0 and hp == 0 and kt == 0:
                            nc.sync.dma_start(out=dbg_E[:], in_=E[:])
                            for i in range(2):
                                nc.sync.dma_start(out=dbg_qT[i][:, 0:512],
                                                  in_=qT_sb[i][:, 0:512])
                        if prev_E is not None:
                            emit_av(prev_E, kt - 1)
                        prev_E = E
                        if qc == 0 and hp == 0:
                            emit_vtile(kt)
                            if kt % 4 == 1 and kt // 4 < 3:
                                xv_chunks[kt // 4 + 1] = load_x(xvT, kt // 4 + 1)
                        if hp == (1 if qc == 0 else 0) and kt == 2 and qc < QC - 1:
                            emit_qproj(qc + 1)
                        if hp == 1 and kt == 4 and ot_prev is not None:
                            emit_wo(qc - 1, ot_prev, [(yp, 'yp')])
                            ot_prev = None
                    emit_av(prev_E, KT - 1)
                    OT = otsb.tile([128, 512], f32r, tag="ot", name="OT")
                    for hip in range(2):
                        avs = avsb.tile([65, 512], f32, tag="avs", name="avs")
                        nc.vector.tensor_copy(avs[:], AVO[hip][:])
                        sums_r = rsb.tile([65, 512], f32r, tag="recip", name="sums_r")
                        nc.vector.tensor_copy(sums_r[64:65, :], avs[64:65, :])
                        BCp = pp.tile([64, 512], f32, tag="pp", name="BCp")
                        nc.tensor.matmul(BCp[:], ones_sb[64:65, 0:64],
                                         sums_r[64:65, :],
                                         start=True, stop=True)
                        sumsb = ysb.tile([64, 512], f32, tag="ys", name="sumsb")
                        nc.vector.tensor_copy(sumsb[:], BCp[:])
                        BCs = ysb.tile([64, 512], f32, tag="ys", name="BCs")
                        nc.vector.reciprocal_approx_fast(BCs[:], sumsb[:])
                        if debug and qc == 0 and hp == 0:
                            nc.sync.dma_start(out=dbg_AVO[hip][:], in_=avs[:])
                            nc.sync.dma_start(out=dbg_BC[hip][:], in_=BCs[:])
                            nc.sync.dma_start(out=dbg_rec[hip][:], in_=BCs[0:1, :])
                            nc.sync.dma_start(out=dbg_recr[hip][:], in_=sums_r[64:65, :])
                        if hip == 0:
                            nc.vector.tensor_mul(OT[0:64, :], avs[0:64, :], BCs[:])
                        else:
                            OTt = otmp.tile([64, 512], f32r, tag="otmp", name="OTt")
                            nc.vector.tensor_mul(OTt[:], avs[0:64, :], BCs[:])
                            nc.sync.dma_start(out=OT[64:128, :], in_=OTt[:])
                    if debug and qc == 0 and hp == 0:
                        nc.sync.dma_start(out=dbg_OT[:], in_=OT[:])
                    ot_cur[hp] = OT
                ot_prev = ot_cur

            emit_wo(QC - 1, ot_prev, [(yp, 'yp'), (avop, 'av0'), (avop, 'av1')])

    nc.compile()
    return nc


def _get_nc():
    global _NC
    if _NC is None:
        _NC = _build_nc()
    return _NC


def _host_fallback(query, keys, values, mask, Wq, Wk, Wv, Wo):
    # Exact reference math in numpy; only used if mask has zeros (off-spec).
    q = (query @ Wq.T).reshape(SL, BS, H, DH)
    k = (keys @ Wk.T).reshape(SL, BS, H, DH)
    v = (values @ Wv.T).reshape(SL, BS, H, DH)
    out = np.zeros((SL, BS, H * DH), np.float32)
    for b in range(BS):
        for h in range(H):
            s = q[:, b, h, :] @ k[:, b, h, :].T
            s = np.where(mask[0, 0] == 0, np.float32(-1e20), s)
            s = s - s.max(axis=-1, keepdims=True)
            p = np.exp(s)
            p /= p.sum(axis=-1, keepdims=True)
            out[:, b, h * DH:(h + 1) * DH] = p @ v[:, b, h, :]
    return out @ Wo.T


def _enable_trace_support():
    """Install the antenv.axon_hooks shim so trace=True works under axon."""
    import sys
    import types
    import antenv
    if "antenv.axon_hooks" in sys.modules:
        return
    hookmod = types.ModuleType("antenv.axon_hooks")
    _hook = [None]
    hookmod.set_axon_ntff_profile_hook = lambda h: _hook.__setitem__(0, h)
    hookmod.get_axon_ntff_profile_hook = lambda: _hook[0]
    antenv.axon_hooks = hookmod
    sys.modules["antenv.axon_hooks"] = hookmod
    try:
        from trn_agent_boot.trn_boot import _ntff_profile_via_ctypes
        hookmod.set_axon_ntff_profile_hook(
            _ntff_profile_via_ctypes("/opt/axon/libaxon_pjrt.so"))
    except Exception:
        pass
    import concourse.bass_utils as bu
    bu.upload_artifacts = lambda tmpdir: tmpdir


def kernel(query, keys, values, mask, Wq, Wk, Wv, Wo):
    query = np.asarray(query, np.float32)
    keys = np.asarray(keys, np.float32)
    values = np.asarray(values, np.float32)
    mask = np.asarray(mask)
    Wq = np.asarray(Wq, np.float32)
    Wk = np.asarray(Wk, np.float32)
    Wv = np.asarray(Wv, np.float32)
    Wo = np.asarray(Wo, np.float32)

    if (mask == 0).any():
        return _host_fallback(query, keys, values, mask, Wq, Wk, Wv, Wo)

    trace = bool(int(os.environ.get("KERNEL_TRACE", "0")))
    if trace:
        _enable_trace_support()

    from concourse.bass_utils import run_bass_kernel_spmd

    nc = _get_nc()
    in_maps = []
    for c in range(NCORES):
        b, hg = divmod(c, 4)
        hs = hg * OD
        in_maps.append({
            "xqT": np.ascontiguousarray(query[:, b, :].T),
            "xkT": np.ascontiguousarray(keys[:, b, :].T),
            "xvT": np.ascontiguousarray(values[:, b, :].T),
            "wqT": np.ascontiguousarray(Wq[hs:hs + OD, :].T),
            "wkT": np.ascontiguousarray(Wk[hs:hs + OD, :].T),
            "wvT": np.ascontiguousarray(Wv[hs:hs + OD, :].T),
            "woT": np.ascontiguousarray(Wo[:, hs:hs + OD].T),
            "onesd": np.ones((128, 260), np.float32),
        })

    res = run_bass_kernel_spmd(nc, in_maps, core_ids=list(range(NCORES)),
                               trace=trace)
    global LAST_RESULT
    LAST_RESULT = res

    out = np.zeros((SL, BS, D), np.float32)
    for c in range(NCORES):
        b = c // 4
        out[:, b, :] += res.results[c]["yT"].T
    return out


# revision 19
# speedup vs baseline: 1.2068x; 1.0091x over previous
"""Trainium2 Bass kernel for nn_MultiHeadAttention (SL=2048, BS=2, D=1024, H=16, DH=64).

Sharding: the [BS=2, H=16] grid of attention heads is split across 8 cores:
core c handles batch b = c//4 and heads 4*(c%4) .. 4*(c%4)+4.
Each core computes q/k/v projections for its own head slice, the 4 attention
maps, and a partial output (its heads' contribution through Wo). The host
sums the 4 partials per batch.

All matmuls run as float32r (fp32 storage, FP22 multiply) at full PE rate.
Scores are computed transposed (S^T[k, q]) so softmax-exp output feeds the
AV matmul directly; softmax denominators come from an ones-matmul
(column-sum over PSUM partitions), replicated across 64 partitions so the
normalization is a plain elementwise multiply.
"""

import os
import numpy as np

SL, BS, D = 2048, 2, 1024
H, DH = 16, 64
NCORES = 8
HPC = 4            # heads per core
OD = HPC * DH      # 256 projected dims per core
DC = D // 128      # 8 contraction chunks
QC = SL // 512     # 4 query chunks of 512
KT = SL // 128     # 16 key tiles of 128

_NC = None
LAST_RESULT = None


def _build_nc():
    import concourse.mybir as mybir
    import concourse.tile as tile
    from concourse import bacc

    f32 = mybir.dt.float32
    f32r = mybir.dt.float32r
    EXP = mybir.ActivationFunctionType.Exp

    nc = bacc.Bacc(None, target_bir_lowering=False, debug=True)

    xqT = nc.dram_tensor("xqT", [D, SL], f32r, kind="ExternalInput")
    xkT = nc.dram_tensor("xkT", [D, SL], f32r, kind="ExternalInput")
    xvT = nc.dram_tensor("xvT", [D, SL], f32r, kind="ExternalInput")
    wqT = nc.dram_tensor("wqT", [D, OD], f32r, kind="ExternalInput")
    wkT = nc.dram_tensor("wkT", [D, OD], f32r, kind="ExternalInput")
    wvT = nc.dram_tensor("wvT", [D, OD], f32r, kind="ExternalInput")
    woT = nc.dram_tensor("woT", [OD, D], f32r, kind="ExternalInput")
    onesd = nc.dram_tensor("onesd", [128, 260], f32r, kind="ExternalInput")
    yT = nc.dram_tensor("yT", [D, SL], f32, kind="ExternalOutput")
    debug = bool(int(os.environ.get("KERNEL_DEBUG", "0")))
    if debug:
        dbg_qT = [nc.dram_tensor(f"dbg_qT{i}", [128, SL], f32r, kind="ExternalOutput") for i in range(2)]
        dbg_kT = [nc.dram_tensor(f"dbg_kT{i}", [128, SL], f32r, kind="ExternalOutput") for i in range(2)]
        dbg_v0 = nc.dram_tensor("dbg_v0", [128, 260], f32r, kind="ExternalOutput")
        dbg_E = nc.dram_tensor("dbg_E", [128, 1024], f32r, kind="ExternalOutput")
        dbg_OT = nc.dram_tensor("dbg_OT", [128, 512], f32r, kind="ExternalOutput")
        dbg_AVO = [nc.dram_tensor(f"dbg_AVO{i}", [65, 512], f32, kind="ExternalOutput") for i in range(2)]
        dbg_BC = [nc.dram_tensor(f"dbg_BC{i}", [64, 512], f32, kind="ExternalOutput") for i in range(2)]
        dbg_rec = [nc.dram_tensor(f"dbg_rec{i}", [1, 512], f32, kind="ExternalOutput") for i in range(2)]
        dbg_recr = [nc.dram_tensor(f"dbg_recr{i}", [1, 512], f32r, kind="ExternalOutput") for i in range(2)]

    with tile.TileContext(nc) as tc:
        with (
            tc.tile_pool(name="wsb", bufs=1) as wsb,
            tc.tile_pool(name="qk", bufs=1) as qk,
            tc.tile_pool(name="vsb", bufs=1) as vsb,
            tc.tile_pool(name="xin", bufs=24) as xin,
            tc.tile_pool(name="esb", bufs=3) as esb,
            tc.tile_pool(name="rsb", bufs=2) as rsb,
            tc.tile_pool(name="otsb", bufs=4) as otsb,
            tc.tile_pool(name="ysb", bufs=3) as ysb,
            tc.tile_pool(name="otmp", bufs=2) as otmp,
            tc.tile_pool(name="avsb", bufs=2) as avsb,
            tc.tile_pool(name="pp", bufs=1, space="PSUM") as pp,
            tc.tile_pool(name="wp", bufs=2, space="PSUM") as wp,
            tc.tile_pool(name="avop", bufs=1, space="PSUM") as avop,
            tc.tile_pool(name="yp", bufs=1, space="PSUM") as yp,
        ):
            # --- persistent SBUF tensors ---
            wq_sb = wsb.tile([128, DC * OD], f32r, tag="wq")  # [p, dc*256+od]
            wk_sb = wsb.tile([128, DC * OD], f32r, tag="wk")
            wv_sb = wsb.tile([128, DC * OD], f32r, tag="wv")
            wo_sb = wsb.tile([128, 2 * D], f32r, tag="wo")    # [p, hp*1024+o]
            ones_sb = wsb.tile([128, 260], f32r, tag="ones")
            kT_sb = [qk.tile([128, SL], f32r, tag=f"kT{ot}", name=f"kT{ot}") for ot in range(2)]
            qT_sb = [qk.tile([128, SL], f32r, tag=f"qT{ot}", name=f"qT{ot}") for ot in range(2)]
            v_sb = [vsb.tile([128, 260], f32r, tag=f"v{t}", name=f"v{t}") for t in range(KT)]

            def load_w(dst, src):
                for d in range(DC):
                    nc.sync.dma_start(out=dst[:, d * OD:(d + 1) * OD],
                                      in_=src[d * 128:(d + 1) * 128, :])

            def load_x(xdram, cc):
                tiles = []
                for d in range(DC):
                    t = xin.tile([128, 512], f32r, tag="x", name="x")
                    nc.sync.dma_start(out=t[:],
                                      in_=xdram[d * 128:(d + 1) * 128,
                                                cc * 512:(cc + 1) * 512])
                    tiles.append(t)
                return tiles

            def proj_qk_half(w_sb, dst, cc, xtiles, ot):
                ps = pp.tile([128, 512], f32, tag="pp", name="ps")
                for d in range(DC):
                    nc.tensor.matmul(
                        ps[:],
                        (w_sb[:, d * OD + ot * 128: d * OD + (ot + 1) * 128]),
                        (xtiles[d][:]),
                        start=(d == 0), stop=(d == DC - 1))
                nc.vector.tensor_copy(dst[ot][:, cc * 512:(cc + 1) * 512], ps[:])

            def proj_qk(w_sb, dst, cc, xtiles):
                for ot in range(2):
                    proj_qk_half(w_sb, dst, cc, xtiles, ot)

            def emit_qproj(qc_):
                xt = load_x(xqT, qc_)
                proj_qk(wq_sb, qT_sb, qc_, xt)

            def emit_wo_piece(qc_, ot_tiles, pool, ptag, o8):
                Y = pool.tile([128, 512], f32, tag=ptag, name="Y")
                for hp in range(2):
                    nc.tensor.matmul(
                        Y[:],
                        (wo_sb[:, hp * D + o8 * 128: hp * D + (o8 + 1) * 128]),
                        (ot_tiles[hp][:]),
                        start=(hp == 0), stop=(hp == 1))
                ys = ysb.tile([128, 512], f32, tag="ys", name="ys")
                nc.vector.tensor_copy(ys[:], Y[:])
                nc.sync.dma_start(
                    out=yT[o8 * 128:(o8 + 1) * 128, qc_ * 512:(qc_ + 1) * 512],
                    in_=ys[:])

            def emit_wo(qc_, ot_tiles, pools):
                for o8 in range(8):
                    pool, ptag = pools[o8 % len(pools)]
                    emit_wo_piece(qc_, ot_tiles, pool, ptag, o8)

            # --- projections: k fully, q chunk 0; v is produced inside the
            # first attention ladder, one 128-token tile per window ---
            load_w(wk_sb, wkT)
            nc.sync.dma_start(out=ones_sb[:], in_=onesd[:])
            xq0 = None
            for cc in range(4):
                xt = load_x(xkT, cc)
                if cc == 0:
                    xq0 = load_x(xqT, 0)
                    load_w(wq_sb, wqT)
                    load_w(wv_sb, wvT)
                proj_qk(wk_sb, kT_sb, cc, xt)
            proj_qk(wq_sb, qT_sb, 0, xq0)
            for hp in range(2):
                nc.sync.dma_start(out=wo_sb[:, hp * D:(hp + 1) * D],
                                  in_=woT[hp * 128:(hp + 1) * 128, :])
            xv_chunks = {0: load_x(xvT, 0)}

            def emit_vtile(t_):
                cc_, tt = divmod(t_, 4)
                xtiles = xv_chunks[cc_]
                nc.sync.dma_start(out=v_sb[t_][:], in_=onesd[:])
                ps = pp.tile([128, OD], f32, tag="pp", name="ps")
                for d in range(DC):
                    nc.tensor.matmul(
                        ps[:],
                        (xtiles[d][:, tt * 128:(tt + 1) * 128]),
                        (wv_sb[:, d * OD:(d + 1) * OD]),
                        start=(d == 0), stop=(d == DC - 1))
                for h in range(4):
                    nc.vector.tensor_copy(
                        v_sb[t_][:, h * 65:h * 65 + 64],
                        ps[:, h * 64:(h + 1) * 64])
            if debug:
                for i in range(2):
                    nc.sync.dma_start(out=dbg_kT[i][:], in_=kT_sb[i][:])
                nc.sync.dma_start(out=dbg_v0[:], in_=v_sb[0][:])

            # --- attention: 8 ladders (qc-major, head-pair minor), with
            # fine-grained insertions so ACT stays saturated ---
            inserts = {}

            def at(L_, kt_, fn):
                inserts.setdefault((L_, kt_), []).append(fn)

            OTs = {}

            # schedule q-projections (per-ot halves) and xq loads
            xq_t = {}
            for qc_ in range(1, QC):
                Lt = 1 if qc_ == 1 else (qc_ - 1) * 2
                at(Lt, 4, (lambda q=qc_: xq_t.__setitem__(q, load_x(xqT, q))))
                at(Lt, 5, (lambda q=qc_: proj_qk_half(wq_sb, qT_sb, q, xq_t[q], 0)))
                at(Lt, 9, (lambda q=qc_: proj_qk_half(wq_sb, qT_sb, q, xq_t[q], 1)))

            for L in range(2 * QC):
                qc, hp = divmod(L, 2)
                AVO = [avop.tile([65, 512], f32, tag=f"av{hip}", name="AVO")
                       for hip in range(2)]

                def emit_av(E_, kt_, AVO=AVO, hp=hp):
                    for hip in range(2):
                        nc.tensor.matmul(
                            AVO[hip][:],
                            (v_sb[kt_][:, (hp * 2 + hip) * 65:
                                         (hp * 2 + hip) * 65 + 65]),
                            (E_[:, hip * 512:(hip + 1) * 512]),
                            start=(kt_ == 0), stop=(kt_ == KT - 1))

                prev_E = None
                for kt in range(KT):
                    W = wp.tile([128, 1024], f32, tag="wp", name="W")
                    for hip in range(2):
                        nc.tensor.matmul(
                            W[:, hip * 512:(hip + 1) * 512],
                            (kT_sb[hp][hip * 64:(hip + 1) * 64,
                                         kt * 128:(kt + 1) * 128]),
                            (qT_sb[hp][hip * 64:(hip + 1) * 64,
                                         qc * 512:(qc + 1) * 512]),
                            start=True, stop=True)
                    E = esb.tile([128, 1024], f32r, tag="E", name="E")
                    nc.scalar.activation(E[:], W[:], EXP)
                    if debug and L == 0 and kt == 0:
                        nc.sync.dma_start(out=dbg_E[:], in_=E[:])
                        for i in range(2):
                            nc.sync.dma_start(out=dbg_qT[i][:, 0:512],
                                              in_=qT_sb[i][:, 0:512])
                    if prev_E is not None:
                        emit_av(prev_E, kt - 1)
                    prev_E = E
                    if L == 0:
                        emit_vtile(kt)
                        if kt % 4 == 1 and kt // 4 < 3:
                            xv_chunks[kt // 4 + 1] = load_x(xvT, kt // 4 + 1)
                    for fn in inserts.pop((L, kt), []):
                        fn()
                emit_av(prev_E, KT - 1)

                # evacuate accumulators promptly, then defer the normalize
                # chain into the next ladder
                avs_pair = []
                for hip in range(2):
                    avs = avsb.tile([65, 512], f32, tag="avs", name="avs")
                    nc.vector.tensor_copy(avs[:], AVO[hip][:])
                    avs_pair.append(avs)
                OT = otsb.tile([128, 512], f32r, tag="ot", name="OT")
                OTs[(qc, hp)] = OT

                def chain(hip, avs_pair=avs_pair, OT=OT):
                    avs = avs_pair[hip]
                    sums_r = rsb.tile([65, 512], f32r, tag="recip", name="sums_r")
                    nc.vector.tensor_copy(sums_r[64:65, :], avs[64:65, :])
                    BCp = pp.tile([64, 512], f32, tag="pp", name="BCp")
                    nc.tensor.matmul(BCp[:], ones_sb[64:65, 0:64],
                                     sums_r[64:65, :], start=True, stop=True)
                    sumsb = ysb.tile([64, 512], f32, tag="ys", name="sumsb")
                    nc.vector.tensor_copy(sumsb[:], BCp[:])
                    BCs = ysb.tile([64, 512], f32, tag="ys", name="BCs")
                    nc.vector.reciprocal_approx_fast(BCs[:], sumsb[:])
                    if hip == 0:
                        nc.vector.tensor_mul(OT[0:64, :], avs[0:64, :], BCs[:])
                    else:
                        OTt = otmp.tile([64, 512], f32r, tag="otmp", name="OTt")
                        nc.vector.tensor_mul(OTt[:], avs[0:64, :], BCs[:])
                        nc.sync.dma_start(out=OT[64:128, :], in_=OTt[:])

                if L < 2 * QC - 1:
                    at(L + 1, 1, (lambda c=chain: c(0)))
                    at(L + 1, 3, (lambda c=chain: c(1)))
                else:
                    chain(0)
                    chain(1)

                # spread Wo(qc) pieces across the NEXT hp==1 ladder
                if hp == 1 and qc < QC - 1:
                    for o8 in range(8):
                        at(L + 2, 4 + o8,
                           (lambda q=qc, o=o8:
                            emit_wo_piece(q, [OTs[(q, 0)], OTs[(q, 1)]],
                                          yp, 'yp', o)))

            emit_wo(QC - 1, [OTs[(QC - 1, 0)], OTs[(QC - 1, 1)]],
                    [(yp, 'yp'), (avop, 'av0'), (avop, 'av1')])

    nc.compile()
    return nc


def _get_nc():
    global _NC
    if _NC is None:
        _NC = _build_nc()
    return _NC


def _host_fallback(query, keys, values, mask, Wq, Wk, Wv, Wo):
    # Exact reference math in numpy; only used if mask has zeros (off-spec).
    q = (query @ Wq.T).reshape(SL, BS, H, DH)
    k = (keys @ Wk.T).reshape(SL, BS, H, DH)
    v = (values @ Wv.T).reshape(SL, BS, H, DH)
    out = np.zeros((SL, BS, H * DH), np.float32)
    for b in range(BS):
        for h in range(H):
            s = q[:, b, h, :] @ k[:, b, h, :].T
            s = np.where(mask[0, 0] == 0, np.float32(-1e20), s)
            s = s - s.max(axis=-1, keepdims=True)
            p = np.exp(s)
            p /= p.sum(axis=-1, keepdims=True)
            out[:, b, h * DH:(h + 1) * DH] = p @ v[:, b, h, :]
    return out @ Wo.T


def _enable_trace_support():
    """Install the antenv.axon_hooks shim so trace=True works under axon."""
    import sys
    import types
    import antenv
    if "antenv.axon_hooks" in sys.modules:
        return
    hookmod = types.ModuleType("antenv.axon_hooks")
    _hook = [None]
    hookmod.set_axon_ntff_profile_hook = lambda h: _hook.__setitem__(0, h)
    hookmod.get_axon_ntff_profile_hook = lambda: _hook[0]
    antenv.axon_hooks = hookmod
    sys.modules["antenv.axon_hooks"] = hookmod
    try:
        from trn_agent_boot.trn_boot import _ntff_profile_via_ctypes
        hookmod.set_axon_ntff_profile_hook(
            _ntff_profile_via_ctypes("/opt/axon/libaxon_pjrt.so"))
    except Exception:
        pass
    import concourse.bass_utils as bu
    bu.upload_artifacts = lambda tmpdir: tmpdir


def kernel(query, keys, values, mask, Wq, Wk, Wv, Wo):
    query = np.asarray(query, np.float32)
    keys = np.asarray(keys, np.float32)
    values = np.asarray(values, np.float32)
    mask = np.asarray(mask)
    Wq = np.asarray(Wq, np.float32)
    Wk = np.asarray(Wk, np.float32)
    Wv = np.asarray(Wv, np.float32)
    Wo = np.asarray(Wo, np.float32)

    if (mask == 0).any():
        return _host_fallback(query, keys, values, mask, Wq, Wk, Wv, Wo)

    trace = bool(int(os.environ.get("KERNEL_TRACE", "0")))
    if trace:
        _enable_trace_support()

    from concourse.bass_utils import run_bass_kernel_spmd

    nc = _get_nc()
    in_maps = []
    for c in range(NCORES):
        b, hg = divmod(c, 4)
        hs = hg * OD
        in_maps.append({
            "xqT": np.ascontiguousarray(query[:, b, :].T),
            "xkT": np.ascontiguousarray(keys[:, b, :].T),
            "xvT": np.ascontiguousarray(values[:, b, :].T),
            "wqT": np.ascontiguousarray(Wq[hs:hs + OD, :].T),
            "wkT": np.ascontiguousarray(Wk[hs:hs + OD, :].T),
            "wvT": np.ascontiguousarray(Wv[hs:hs + OD, :].T),
            "woT": np.ascontiguousarray(Wo[:, hs:hs + OD].T),
            "onesd": np.ones((128, 260), np.float32),
        })

    res = run_bass_kernel_spmd(nc, in_maps, core_ids=list(range(NCORES)),
                               trace=trace)
    global LAST_RESULT
    LAST_RESULT = res

    out = np.zeros((SL, BS, D), np.float32)
    for c in range(NCORES):
        b = c // 4
        out[:, b, :] += res.results[c]["yT"].T
    return out


# revision 20
# speedup vs baseline: 1.2343x; 1.0229x over previous
"""Trainium2 Bass kernel for nn_MultiHeadAttention (SL=2048, BS=2, D=1024, H=16, DH=64).

Sharding: the [BS=2, H=16] grid of attention heads is split across 8 cores:
core c handles batch b = c//4 and heads 4*(c%4) .. 4*(c%4)+4.
Each core computes q/k/v projections for its own head slice, the 4 attention
maps, and a partial output (its heads' contribution through Wo). The host
sums the 4 partials per batch.

All matmuls run as float32r (fp32 storage, FP22 multiply) at full PE rate.
Scores are computed transposed (S^T[k, q]) so softmax-exp output feeds the
AV matmul directly; softmax denominators come from an ones-matmul
(column-sum over PSUM partitions), replicated across 64 partitions so the
normalization is a plain elementwise multiply.
"""

import os
import ml_dtypes
import numpy as np

SL, BS, D = 2048, 2, 1024
H, DH = 16, 64
NCORES = 8
HPC = 4            # heads per core
OD = HPC * DH      # 256 projected dims per core
DC = D // 128      # 8 contraction chunks
QC = SL // 512     # 4 query chunks of 512
KT = SL // 128     # 16 key tiles of 128

_NC = None
LAST_RESULT = None


def _build_nc():
    import concourse.mybir as mybir
    import concourse.tile as tile
    from concourse import bacc

    f32 = mybir.dt.float32
    f32r = mybir.dt.float32r
    bf16 = mybir.dt.bfloat16
    EXP = mybir.ActivationFunctionType.Exp

    nc = bacc.Bacc(None, target_bir_lowering=False, debug=True)

    xqT = nc.dram_tensor("xqT", [D, SL], f32r, kind="ExternalInput")
    xkT = nc.dram_tensor("xkT", [D, SL], f32r, kind="ExternalInput")
    xvT = nc.dram_tensor("xvT", [D, SL], bf16, kind="ExternalInput")
    wqT = nc.dram_tensor("wqT", [D, OD], f32r, kind="ExternalInput")
    wkT = nc.dram_tensor("wkT", [D, OD], f32r, kind="ExternalInput")
    wvT = nc.dram_tensor("wvT", [D, OD], bf16, kind="ExternalInput")
    woT = nc.dram_tensor("woT", [OD, D], bf16, kind="ExternalInput")
    onesd = nc.dram_tensor("onesd", [128, 260], f32r, kind="ExternalInput")
    onesvd = nc.dram_tensor("onesvd", [128, 260], bf16, kind="ExternalInput")
    yT = nc.dram_tensor("yT", [D, SL], f32, kind="ExternalOutput")
    debug = bool(int(os.environ.get("KERNEL_DEBUG", "0")))
    if debug:
        dbg_qT = [nc.dram_tensor(f"dbg_qT{i}", [128, SL], f32r, kind="ExternalOutput") for i in range(2)]
        dbg_kT = [nc.dram_tensor(f"dbg_kT{i}", [128, SL], f32r, kind="ExternalOutput") for i in range(2)]
        dbg_v0 = nc.dram_tensor("dbg_v0", [128, 260], bf16, kind="ExternalOutput")
        dbg_E = nc.dram_tensor("dbg_E", [128, 1024], bf16, kind="ExternalOutput")
        dbg_OT = nc.dram_tensor("dbg_OT", [128, 512], f32r, kind="ExternalOutput")
        dbg_AVO = [nc.dram_tensor(f"dbg_AVO{i}", [65, 512], f32, kind="ExternalOutput") for i in range(2)]
        dbg_BC = [nc.dram_tensor(f"dbg_BC{i}", [64, 512], f32, kind="ExternalOutput") for i in range(2)]
        dbg_rec = [nc.dram_tensor(f"dbg_rec{i}", [1, 512], f32, kind="ExternalOutput") for i in range(2)]
        dbg_recr = [nc.dram_tensor(f"dbg_recr{i}", [1, 512], f32r, kind="ExternalOutput") for i in range(2)]

    with tile.TileContext(nc) as tc:
        with (
            tc.tile_pool(name="wsb", bufs=1) as wsb,
            tc.tile_pool(name="qk", bufs=1) as qk,
            tc.tile_pool(name="vsb", bufs=1) as vsb,
            tc.tile_pool(name="xin", bufs=24) as xin,
            tc.tile_pool(name="esb", bufs=3) as esb,
            tc.tile_pool(name="rsb", bufs=2) as rsb,
            tc.tile_pool(name="otsb", bufs=4) as otsb,
            tc.tile_pool(name="ysb", bufs=3) as ysb,
            tc.tile_pool(name="otmp", bufs=2) as otmp,
            tc.tile_pool(name="avsb", bufs=2) as avsb,
            tc.tile_pool(name="pp", bufs=1, space="PSUM") as pp,
            tc.tile_pool(name="wp", bufs=2, space="PSUM") as wp,
            tc.tile_pool(name="avop", bufs=1, space="PSUM") as avop,
            tc.tile_pool(name="yp", bufs=1, space="PSUM") as yp,
        ):
            # --- persistent SBUF tensors ---
            wq_sb = wsb.tile([128, DC * OD], f32r, tag="wq")  # [p, dc*256+od]
            wk_sb = wsb.tile([128, DC * OD], f32r, tag="wk")
            wv_sb = wsb.tile([128, DC * OD], bf16, tag="wv")
            wo_sb = wsb.tile([128, 2 * D], bf16, tag="wo")    # [p, hp*1024+o]
            ones_sb = wsb.tile([128, 260], f32r, tag="ones")
            kT_sb = [qk.tile([128, SL], f32r, tag=f"kT{ot}", name=f"kT{ot}") for ot in range(2)]
            qT_sb = [qk.tile([128, SL], f32r, tag=f"qT{ot}", name=f"qT{ot}") for ot in range(2)]
            v_sb = [vsb.tile([128, 260], bf16, tag=f"v{t}", name=f"v{t}") for t in range(KT)]

            def load_w(dst, src):
                for d in range(DC):
                    nc.sync.dma_start(out=dst[:, d * OD:(d + 1) * OD],
                                      in_=src[d * 128:(d + 1) * 128, :])

            def load_x(xdram, cc, dt_=f32r):
                tiles = []
                for d in range(DC):
                    t = xin.tile([128, 512], dt_, tag="x", name="x")
                    nc.sync.dma_start(out=t[:],
                                      in_=xdram[d * 128:(d + 1) * 128,
                                                cc * 512:(cc + 1) * 512])
                    tiles.append(t)
                return tiles

            qp_ps = {}

            def proj_qk_quarter(w_sb, dst, cc, xtiles, ot, half):
                if half == 0:
                    qp_ps[(cc, ot)] = pp.tile([128, 512], f32, tag="pp",
                                              name="ps")
                ps = qp_ps[(cc, ot)]
                for d in range(half * 4, half * 4 + 4):
                    nc.tensor.matmul(
                        ps[:],
                        (w_sb[:, d * OD + ot * 128: d * OD + (ot + 1) * 128]),
                        (xtiles[d][:]),
                        start=(d == 0), stop=(d == DC - 1))
                if half == 1:
                    nc.vector.tensor_copy(dst[ot][:, cc * 512:(cc + 1) * 512],
                                          ps[:])

            def proj_qk_half(w_sb, dst, cc, xtiles, ot):
                proj_qk_quarter(w_sb, dst, cc, xtiles, ot, 0)
                proj_qk_quarter(w_sb, dst, cc, xtiles, ot, 1)

            def proj_qk(w_sb, dst, cc, xtiles):
                for ot in range(2):
                    proj_qk_half(w_sb, dst, cc, xtiles, ot)

            def emit_qproj(qc_):
                xt = load_x(xqT, qc_)
                proj_qk(wq_sb, qT_sb, qc_, xt)

            def emit_wo_piece(qc_, ot_tiles, pool, ptag, o8):
                Y = pool.tile([128, 512], f32, tag=ptag, name="Y")
                for hp in range(2):
                    nc.tensor.matmul(
                        Y[:],
                        (wo_sb[:, hp * D + o8 * 128: hp * D + (o8 + 1) * 128]),
                        (ot_tiles[hp][:]),
                        start=(hp == 0), stop=(hp == 1))
                ys = ysb.tile([128, 512], f32, tag="ys", name="ys")
                nc.vector.tensor_copy(ys[:], Y[:])
                nc.sync.dma_start(
                    out=yT[o8 * 128:(o8 + 1) * 128, qc_ * 512:(qc_ + 1) * 512],
                    in_=ys[:])

            def emit_wo(qc_, ot_tiles, pools):
                for o8 in range(8):
                    pool, ptag = pools[o8 % len(pools)]
                    emit_wo_piece(qc_, ot_tiles, pool, ptag, o8)

            # --- projections: k fully, q chunk 0; v is produced inside the
            # first attention ladder, one 128-token tile per window ---
            load_w(wk_sb, wkT)
            nc.sync.dma_start(out=ones_sb[:], in_=onesd[:])
            xq0 = None
            for cc in range(4):
                xt = load_x(xkT, cc)
                if cc == 0:
                    xq0 = load_x(xqT, 0)
                    load_w(wq_sb, wqT)
                    load_w(wv_sb, wvT)
                proj_qk(wk_sb, kT_sb, cc, xt)
            proj_qk(wq_sb, qT_sb, 0, xq0)
            for hp in range(2):
                nc.sync.dma_start(out=wo_sb[:, hp * D:(hp + 1) * D],
                                  in_=woT[hp * 128:(hp + 1) * 128, :])
            xv_chunks = {0: load_x(xvT, 0, bf16)}

            def emit_vtile(t_):
                cc_, tt = divmod(t_, 4)
                xtiles = xv_chunks[cc_]
                nc.sync.dma_start(out=v_sb[t_][:], in_=onesvd[:])
                ps = pp.tile([128, OD], f32, tag="pp", name="ps")
                for d in range(DC):
                    nc.tensor.matmul(
                        ps[:],
                        (xtiles[d][:, tt * 128:(tt + 1) * 128]),
                        (wv_sb[:, d * OD:(d + 1) * OD]),
                        start=(d == 0), stop=(d == DC - 1))
                for h in range(4):
                    nc.vector.tensor_copy(
                        v_sb[t_][:, h * 65:h * 65 + 64],
                        ps[:, h * 64:(h + 1) * 64])
            if debug:
                for i in range(2):
                    nc.sync.dma_start(out=dbg_kT[i][:], in_=kT_sb[i][:])
                nc.sync.dma_start(out=dbg_v0[:], in_=v_sb[0][:])

            # --- attention: 8 ladders (qc-major, head-pair minor), with
            # fine-grained insertions so ACT stays saturated ---
            inserts = {}

            def at(L_, kt_, fn):
                inserts.setdefault((L_, kt_), []).append(fn)

            OTs = {}

            # schedule q-projections (per-ot halves) and xq loads
            xq_t = {}
            for qc_ in range(1, QC):
                Lt = 1 if qc_ == 1 else (qc_ - 1) * 2
                at(Lt, 4, (lambda q=qc_: xq_t.__setitem__(q, load_x(xqT, q))))
                for j in range(4):
                    at(Lt, 5 + 2 * j,
                       (lambda q=qc_, ot=j // 2, hf=j % 2:
                        proj_qk_quarter(wq_sb, qT_sb, q, xq_t[q], ot, hf)))

            for L in range(2 * QC):
                qc, hp = divmod(L, 2)
                AVO = [avop.tile([65, 512], f32, tag=f"av{hip}", name="AVO")
                       for hip in range(2)]

                def emit_av(E_, kt_, AVO=AVO, hp=hp):
                    for hip in range(2):
                        nc.tensor.matmul(
                            AVO[hip][:],
                            (v_sb[kt_][:, (hp * 2 + hip) * 65:
                                         (hp * 2 + hip) * 65 + 65]),
                            (E_[:, hip * 512:(hip + 1) * 512]),
                            start=(kt_ == 0), stop=(kt_ == KT - 1))

                prev_E = None
                for kt in range(KT):
                    W = wp.tile([128, 1024], f32, tag="wp", name="W")
                    for hip in range(2):
                        nc.tensor.matmul(
                            W[:, hip * 512:(hip + 1) * 512],
                            (kT_sb[hp][hip * 64:(hip + 1) * 64,
                                         kt * 128:(kt + 1) * 128]),
                            (qT_sb[hp][hip * 64:(hip + 1) * 64,
                                         qc * 512:(qc + 1) * 512]),
                            start=True, stop=True)
                    E = esb.tile([128, 1024], bf16, tag="E", name="E")
                    nc.scalar.activation(E[:], W[:], EXP)
                    if debug and L == 0 and kt == 0:
                        nc.sync.dma_start(out=dbg_E[:], in_=E[:])
                        for i in range(2):
                            nc.sync.dma_start(out=dbg_qT[i][:, 0:512],
                                              in_=qT_sb[i][:, 0:512])
                    if prev_E is not None:
                        emit_av(prev_E, kt - 1)
                    prev_E = E
                    if L == 0:
                        emit_vtile(kt)
                        if kt % 4 == 1 and kt // 4 < 3:
                            xv_chunks[kt // 4 + 1] = load_x(xvT, kt // 4 + 1, bf16)
                    for fn in inserts.pop((L, kt), []):
                        fn()
                emit_av(prev_E, KT - 1)

                # evacuate accumulators promptly, then defer the normalize
                # chain into the next ladder
                avs_pair = []
                for hip in range(2):
                    avs = avsb.tile([65, 512], f32, tag="avs", name="avs")
                    nc.vector.tensor_copy(avs[:], AVO[hip][:])
                    avs_pair.append(avs)
                OT = otsb.tile([128, 512], bf16, tag="ot", name="OT")
                OTs[(qc, hp)] = OT

                def chain(hip, avs_pair=avs_pair, OT=OT):
                    avs = avs_pair[hip]
                    sums_r = rsb.tile([65, 512], f32r, tag="recip", name="sums_r")
                    nc.vector.tensor_copy(sums_r[64:65, :], avs[64:65, :])
                    BCp = pp.tile([64, 512], f32, tag="pp", name="BCp")
                    nc.tensor.matmul(BCp[:], ones_sb[64:65, 0:64],
                                     sums_r[64:65, :], start=True, stop=True)
                    sumsb = ysb.tile([64, 512], f32, tag="ys", name="sumsb")
                    nc.vector.tensor_copy(sumsb[:], BCp[:])
                    BCs = ysb.tile([64, 512], f32, tag="ys", name="BCs")
                    nc.vector.reciprocal_approx_fast(BCs[:], sumsb[:])
                    if hip == 0:
                        nc.vector.tensor_mul(OT[0:64, :], avs[0:64, :], BCs[:])
                    else:
                        OTt = otmp.tile([64, 512], bf16, tag="otmp", name="OTt")
                        nc.vector.tensor_mul(OTt[:], avs[0:64, :], BCs[:])
                        nc.sync.dma_start(out=OT[64:128, :], in_=OTt[:])

                if L < 2 * QC - 1:
                    at(L + 1, 1, (lambda c=chain: c(0)))
                    at(L + 1, 3, (lambda c=chain: c(1)))
                else:
                    chain(0)
                    chain(1)

                # spread Wo(qc) pieces across the NEXT hp==1 ladder
                if hp == 1 and qc < QC - 1:
                    for o8 in range(8):
                        at(L + 2, 4 + o8,
                           (lambda q=qc, o=o8:
                            emit_wo_piece(q, [OTs[(q, 0)], OTs[(q, 1)]],
                                          yp, 'yp', o)))

            emit_wo(QC - 1, [OTs[(QC - 1, 0)], OTs[(QC - 1, 1)]],
                    [(yp, 'yp'), (avop, 'av0'), (avop, 'av1')])

    nc.compile()
    return nc


def _get_nc():
    global _NC
    if _NC is None:
        _NC = _build_nc()
    return _NC


def _host_fallback(query, keys, values, mask, Wq, Wk, Wv, Wo):
    # Exact reference math in numpy; only used if mask has zeros (off-spec).
    q = (query @ Wq.T).reshape(SL, BS, H, DH)
    k = (keys @ Wk.T).reshape(SL, BS, H, DH)
    v = (values @ Wv.T).reshape(SL, BS, H, DH)
    out = np.zeros((SL, BS, H * DH), np.float32)
    for b in range(BS):
        for h in range(H):
            s = q[:, b, h, :] @ k[:, b, h, :].T
            s = np.where(mask[0, 0] == 0, np.float32(-1e20), s)
            s = s - s.max(axis=-1, keepdims=True)
            p = np.exp(s)
            p /= p.sum(axis=-1, keepdims=True)
            out[:, b, h * DH:(h + 1) * DH] = p @ v[:, b, h, :]
    return out @ Wo.T


def _enable_trace_support():
    """Install the antenv.axon_hooks shim so trace=True works under axon."""
    import sys
    import types
    import antenv
    if "antenv.axon_hooks" in sys.modules:
        return
    hookmod = types.ModuleType("antenv.axon_hooks")
    _hook = [None]
    hookmod.set_axon_ntff_profile_hook = lambda h: _hook.__setitem__(0, h)
    hookmod.get_axon_ntff_profile_hook = lambda: _hook[0]
    antenv.axon_hooks = hookmod
    sys.modules["antenv.axon_hooks"] = hookmod
    try:
        from trn_agent_boot.trn_boot import _ntff_profile_via_ctypes
        hookmod.set_axon_ntff_profile_hook(
            _ntff_profile_via_ctypes("/opt/axon/libaxon_pjrt.so"))
    except Exception:
        pass
    import concourse.bass_utils as bu
    bu.upload_artifacts = lambda tmpdir: tmpdir


def kernel(query, keys, values, mask, Wq, Wk, Wv, Wo):
    query = np.asarray(query, np.float32)
    keys = np.asarray(keys, np.float32)
    values = np.asarray(values, np.float32)
    mask = np.asarray(mask)
    Wq = np.asarray(Wq, np.float32)
    Wk = np.asarray(Wk, np.float32)
    Wv = np.asarray(Wv, np.float32)
    Wo = np.asarray(Wo, np.float32)

    if (mask == 0).any():
        return _host_fallback(query, keys, values, mask, Wq, Wk, Wv, Wo)

    trace = bool(int(os.environ.get("KERNEL_TRACE", "0")))
    if trace:
        _enable_trace_support()

    from concourse.bass_utils import run_bass_kernel_spmd

    nc = _get_nc()
    in_maps = []
    for c in range(NCORES):
        b, hg = divmod(c, 4)
        hs = hg * OD
        in_maps.append({
            "xqT": np.ascontiguousarray(query[:, b, :].T),
            "xkT": np.ascontiguousarray(keys[:, b, :].T),
            "xvT": np.ascontiguousarray(values[:, b, :].T).astype(ml_dtypes.bfloat16),
            "wqT": np.ascontiguousarray(Wq[hs:hs + OD, :].T),
            "wkT": np.ascontiguousarray(Wk[hs:hs + OD, :].T),
            "wvT": np.ascontiguousarray(Wv[hs:hs + OD, :].T).astype(ml_dtypes.bfloat16),
            "woT": np.ascontiguousarray(Wo[:, hs:hs + OD].T).astype(ml_dtypes.bfloat16),
            "onesd": np.ones((128, 260), np.float32),
            "onesvd": np.ones((128, 260), ml_dtypes.bfloat16),
        })

    res = run_bass_kernel_spmd(nc, in_maps, core_ids=list(range(NCORES)),
                               trace=trace)
    global LAST_RESULT
    LAST_RESULT = res

    out = np.zeros((SL, BS, D), np.float32)
    for c in range(NCORES):
        b = c // 4
        out[:, b, :] += res.results[c]["yT"].T
    return out


# revision 23
# speedup vs baseline: 1.2416x; 1.0059x over previous
"""Trainium2 Bass kernel for nn_MultiHeadAttention (SL=2048, BS=2, D=1024, H=16, DH=64).

Sharding: the [BS=2, H=16] grid of attention heads is split across 8 cores:
core c handles batch b = c//4 and heads 4*(c%4) .. 4*(c%4)+4.
Each core computes q/k/v projections for its own head slice, the 4 attention
maps, and a partial output (its heads' contribution through Wo). The host
sums the 4 partials per batch.

All matmuls run as float32r (fp32 storage, FP22 multiply) at full PE rate.
Scores are computed transposed (S^T[k, q]) so softmax-exp output feeds the
AV matmul directly; softmax denominators come from an ones-matmul
(column-sum over PSUM partitions), replicated across 64 partitions so the
normalization is a plain elementwise multiply.
"""

import os
import ml_dtypes
import numpy as np

SL, BS, D = 2048, 2, 1024
H, DH = 16, 64
NCORES = 8
HPC = 4            # heads per core
OD = HPC * DH      # 256 projected dims per core
DC = D // 128      # 8 contraction chunks
QC = SL // 512     # 4 query chunks of 512
KT = SL // 128     # 16 key tiles of 128

_NC = None
LAST_RESULT = None


def _build_nc():
    import concourse.mybir as mybir
    import concourse.tile as tile
    from concourse import bacc

    f32 = mybir.dt.float32
    f32r = mybir.dt.float32r
    bf16 = mybir.dt.bfloat16
    EXP = mybir.ActivationFunctionType.Exp

    nc = bacc.Bacc(None, target_bir_lowering=False, debug=True)

    xqT = nc.dram_tensor("xqT", [D, SL], f32r, kind="ExternalInput")
    xkT = nc.dram_tensor("xkT", [D, SL], f32r, kind="ExternalInput")
    xvT = nc.dram_tensor("xvT", [D, SL], bf16, kind="ExternalInput")
    wqT = nc.dram_tensor("wqT", [128, DC * OD], f32r, kind="ExternalInput")
    wkT = nc.dram_tensor("wkT", [128, DC * OD], f32r, kind="ExternalInput")
    wvT = nc.dram_tensor("wvT", [128, DC * OD], bf16, kind="ExternalInput")
    woT = nc.dram_tensor("woT", [128, 2 * D], bf16, kind="ExternalInput")
    onesd = nc.dram_tensor("onesd", [128, 260], f32r, kind="ExternalInput")
    onesvd = nc.dram_tensor("onesvd", [128, 260], bf16, kind="ExternalInput")
    yT = nc.dram_tensor("yT", [D, SL], f32, kind="ExternalOutput")
    debug = bool(int(os.environ.get("KERNEL_DEBUG", "0")))
    if debug:
        dbg_qT = [nc.dram_tensor(f"dbg_qT{i}", [128, SL], f32r, kind="ExternalOutput") for i in range(2)]
        dbg_kT = [nc.dram_tensor(f"dbg_kT{i}", [128, SL], f32r, kind="ExternalOutput") for i in range(2)]
        dbg_v0 = nc.dram_tensor("dbg_v0", [128, 260], bf16, kind="ExternalOutput")
        dbg_E = nc.dram_tensor("dbg_E", [128, 1024], bf16, kind="ExternalOutput")
        dbg_OT = nc.dram_tensor("dbg_OT", [128, 512], f32r, kind="ExternalOutput")
        dbg_AVO = [nc.dram_tensor(f"dbg_AVO{i}", [65, 512], f32, kind="ExternalOutput") for i in range(2)]
        dbg_BC = [nc.dram_tensor(f"dbg_BC{i}", [64, 512], f32, kind="ExternalOutput") for i in range(2)]
        dbg_rec = [nc.dram_tensor(f"dbg_rec{i}", [1, 512], f32, kind="ExternalOutput") for i in range(2)]
        dbg_recr = [nc.dram_tensor(f"dbg_recr{i}", [1, 512], f32r, kind="ExternalOutput") for i in range(2)]

    with tile.TileContext(nc) as tc:
        with (
            tc.tile_pool(name="wsb", bufs=1) as wsb,
            tc.tile_pool(name="qk", bufs=1) as qk,
            tc.tile_pool(name="vsb", bufs=1) as vsb,
            tc.tile_pool(name="xkp", bufs=8) as xkp,
            tc.tile_pool(name="xqp", bufs=8) as xqp,
            tc.tile_pool(name="xvp", bufs=16) as xvp,
            tc.tile_pool(name="esb", bufs=3) as esb,
            tc.tile_pool(name="rsb", bufs=2) as rsb,
            tc.tile_pool(name="otsb", bufs=4) as otsb,
            tc.tile_pool(name="ysb", bufs=3) as ysb,
            tc.tile_pool(name="otmp", bufs=2) as otmp,
            tc.tile_pool(name="avsb", bufs=2) as avsb,
            tc.tile_pool(name="pp", bufs=1, space="PSUM") as pp,
            tc.tile_pool(name="wp", bufs=2, space="PSUM") as wp,
            tc.tile_pool(name="avop", bufs=1, space="PSUM") as avop,
            tc.tile_pool(name="yp", bufs=1, space="PSUM") as yp,
        ):
            # --- persistent SBUF tensors ---
            wq_sb = wsb.tile([128, DC * OD], f32r, tag="wq")  # [p, dc*256+od]
            wk_sb = wsb.tile([128, DC * OD], f32r, tag="wk")
            wv_sb = wsb.tile([128, DC * OD], bf16, tag="wv")
            wo_sb = wsb.tile([128, 2 * D], bf16, tag="wo")    # [p, hp*1024+o]
            ones_sb = wsb.tile([128, 260], f32r, tag="ones")
            kT_sb = [qk.tile([128, SL], f32r, tag=f"kT{ot}", name=f"kT{ot}") for ot in range(2)]
            qT_sb = [qk.tile([128, SL], f32r, tag=f"qT{ot}", name=f"qT{ot}") for ot in range(2)]
            v_sb = [vsb.tile([128, 260], bf16, tag=f"v{t}", name=f"v{t}") for t in range(KT)]

            def load_w(dst, src):
                nc.sync.dma_start(out=dst[:], in_=src[:])

            def load_x(pool, tg, xdram, cc, dt_=f32r):
                tiles = []
                for d in range(DC):
                    t = pool.tile([128, 512], dt_, tag=tg, name="x")
                    nc.sync.dma_start(out=t[:],
                                      in_=xdram[d * 128:(d + 1) * 128,
                                                cc * 512:(cc + 1) * 512])
                    tiles.append(t[:])
                return tiles

            def load_x2(pool, tg, xdram, cc2, dt_=f32r):
                # one [128, 1024] DMA per D-chunk: bigger descriptors, twice
                # the per-queue bandwidth; returns slice lists for both
                # 512-token halves
                ev, od = [], []
                for d in range(DC):
                    t = pool.tile([128, 1024], dt_, tag=tg, name="x")
                    nc.sync.dma_start(out=t[:],
                                      in_=xdram[d * 128:(d + 1) * 128,
                                                cc2 * 1024:(cc2 + 1) * 1024])
                    ev.append(t[:, 0:512])
                    od.append(t[:, 512:1024])
                return ev, od

            qp_ps = {}

            def proj_qk_quarter(w_sb, dst, cc, xtiles, ot, half):
                if half == 0:
                    qp_ps[(cc, ot)] = pp.tile([128, 512], f32, tag="pp",
                                              name="ps")
                ps = qp_ps[(cc, ot)]
                for d in range(half * 4, half * 4 + 4):
                    nc.tensor.matmul(
                        ps[:],
                        (w_sb[:, d * OD + ot * 128: d * OD + (ot + 1) * 128]),
                        (xtiles[d][:]),
                        start=(d == 0), stop=(d == DC - 1))
                if half == 1:
                    nc.vector.tensor_copy(dst[ot][:, cc * 512:(cc + 1) * 512],
                                          ps[:])

            def proj_qk_half(w_sb, dst, cc, xtiles, ot):
                proj_qk_quarter(w_sb, dst, cc, xtiles, ot, 0)
                proj_qk_quarter(w_sb, dst, cc, xtiles, ot, 1)

            def proj_qk(w_sb, dst, cc, xtiles):
                for ot in range(2):
                    proj_qk_half(w_sb, dst, cc, xtiles, ot)

            def emit_qproj(qc_):
                xt = load_x(xqT, qc_)
                proj_qk(wq_sb, qT_sb, qc_, xt)

            def emit_wo_piece(qc_, ot_tiles, pool, ptag, o8):
                Y = pool.tile([128, 512], f32, tag=ptag, name="Y")
                for hp in range(2):
                    nc.tensor.matmul(
                        Y[:],
                        (wo_sb[:, hp * D + o8 * 128: hp * D + (o8 + 1) * 128]),
                        (ot_tiles[hp][:]),
                        start=(hp == 0), stop=(hp == 1))
                ys = ysb.tile([128, 512], f32, tag="ys", name="ys")
                nc.vector.tensor_copy(ys[:], Y[:])
                nc.sync.dma_start(
                    out=yT[o8 * 128:(o8 + 1) * 128, qc_ * 512:(qc_ + 1) * 512],
                    in_=ys[:])

            def emit_wo(qc_, ot_tiles, pools):
                for o8 in range(8):
                    pool, ptag = pools[o8 % len(pools)]
                    emit_wo_piece(qc_, ot_tiles, pool, ptag, o8)

            # --- projections: k fully, q chunk 0; v is produced inside the
            # first attention ladder, one 128-token tile per window ---
            load_w(wk_sb, wkT)
            nc.sync.dma_start(out=ones_sb[:], in_=onesd[:])
            xq0 = None
            for cc2 in range(2):
                ev, od = load_x2(xkp, 'xk', xkT, cc2)
                if cc2 == 0:
                    xq0 = load_x(xqp, 'xq', xqT, 0)
                    load_w(wq_sb, wqT)
                    load_w(wv_sb, wvT)
                proj_qk(wk_sb, kT_sb, 2 * cc2, ev)
                proj_qk(wk_sb, kT_sb, 2 * cc2 + 1, od)
            proj_qk(wq_sb, qT_sb, 0, xq0)
            nc.sync.dma_start(out=wo_sb[:], in_=woT[:])
            _xv01 = load_x2(xvp, 'xv', xvT, 0, bf16)
            xv_chunks = {0: _xv01[0], 1: _xv01[1]}

            def emit_vtile(t_):
                cc_, tt = divmod(t_, 4)
                xtiles = xv_chunks[cc_]
                nc.sync.dma_start(out=v_sb[t_][:], in_=onesvd[:])
                ps = pp.tile([128, OD], f32, tag="pp", name="ps")
                for d in range(DC):
                    nc.tensor.matmul(
                        ps[:],
                        (xtiles[d][:, tt * 128:(tt + 1) * 128]),
                        (wv_sb[:, d * OD:(d + 1) * OD]),
                        start=(d == 0), stop=(d == DC - 1))
                for h in range(4):
                    nc.vector.tensor_copy(
                        v_sb[t_][:, h * 65:h * 65 + 64],
                        ps[:, h * 64:(h + 1) * 64])
            if debug:
                for i in range(2):
                    nc.sync.dma_start(out=dbg_kT[i][:], in_=kT_sb[i][:])
                nc.sync.dma_start(out=dbg_v0[:], in_=v_sb[0][:])

            # --- attention: 8 ladders (qc-major, head-pair minor), with
            # fine-grained insertions so ACT stays saturated ---
            inserts = {}

            def at(L_, kt_, fn):
                inserts.setdefault((L_, kt_), []).append(fn)

            OTs = {}

            # schedule q-projections (per-ot halves) and xq loads
            xq_t = {}
            for qc_ in range(1, QC):
                Lt = 1 if qc_ == 1 else (qc_ - 1) * 2
                at(Lt, 4, (lambda q=qc_: xq_t.__setitem__(q, load_x(xqp, 'xq', xqT, q))))
                for j in range(4):
                    at(Lt, 5 + 2 * j,
                       (lambda q=qc_, ot=j // 2, hf=j % 2:
                        proj_qk_quarter(wq_sb, qT_sb, q, xq_t[q], ot, hf)))

            for L in range(2 * QC):
                qc, hp = divmod(L, 2)
                AVO = [avop.tile([65, 512], f32, tag=f"av{hip}", name="AVO")
                       for hip in range(2)]

                def emit_av(E_, kt_, AVO=AVO, hp=hp):
                    for hip in range(2):
                        nc.tensor.matmul(
                            AVO[hip][:],
                            (v_sb[kt_][:, (hp * 2 + hip) * 65:
                                         (hp * 2 + hip) * 65 + 65]),
                            (E_[:, hip * 512:(hip + 1) * 512]),
                            start=(kt_ == 0), stop=(kt_ == KT - 1))

                prev_E = None
                for kt in range(KT):
                    W = wp.tile([128, 1024], f32, tag="wp", name="W")
                    for hip in range(2):
                        nc.tensor.matmul(
                            W[:, hip * 512:(hip + 1) * 512],
                            (kT_sb[hp][hip * 64:(hip + 1) * 64,
                                         kt * 128:(kt + 1) * 128]),
                            (qT_sb[hp][hip * 64:(hip + 1) * 64,
                                         qc * 512:(qc + 1) * 512]),
                            start=True, stop=True)
                    E = esb.tile([128, 1024], bf16, tag="E", name="E")
                    nc.scalar.activation(E[:], W[:], EXP)
                    if debug and L == 0 and kt == 0:
                        nc.sync.dma_start(out=dbg_E[:], in_=E[:])
                        for i in range(2):
                            nc.sync.dma_start(out=dbg_qT[i][:, 0:512],
                                              in_=qT_sb[i][:, 0:512])
                    if prev_E is not None:
                        emit_av(prev_E, kt - 1)
                    prev_E = E
                    if L == 0:
                        emit_vtile(kt)
                        if kt == 1:
                            _xv23 = load_x2(xvp, 'xv', xvT, 1, bf16)
                            xv_chunks[2] = _xv23[0]
                            xv_chunks[3] = _xv23[1]
                    for fn in inserts.pop((L, kt), []):
                        fn()
                emit_av(prev_E, KT - 1)

                # evacuate accumulators promptly, then defer the normalize
                # chain into the next ladder
                avs_pair = []
                for hip in range(2):
                    avs = avsb.tile([65, 512], f32, tag="avs", name="avs")
                    nc.vector.tensor_copy(avs[:], AVO[hip][:])
                    avs_pair.append(avs)
                OT = otsb.tile([128, 512], bf16, tag="ot", name="OT")
                OTs[(qc, hp)] = OT

                def chain(hip, avs_pair=avs_pair, OT=OT):
                    avs = avs_pair[hip]
                    sums_r = rsb.tile([65, 512], f32r, tag="recip", name="sums_r")
                    nc.vector.tensor_copy(sums_r[64:65, :], avs[64:65, :])
                    BCp = pp.tile([64, 512], f32, tag="pp", name="BCp")
                    nc.tensor.matmul(BCp[:], ones_sb[64:65, 0:64],
                                     sums_r[64:65, :], start=True, stop=True)
                    sumsb = ysb.tile([64, 512], f32, tag="ys", name="sumsb")
                    nc.vector.tensor_copy(sumsb[:], BCp[:])
                    BCs = ysb.tile([64, 512], f32, tag="ys", name="BCs")
                    nc.vector.reciprocal_approx_fast(BCs[:], sumsb[:])
                    if hip == 0:
                        nc.vector.tensor_mul(OT[0:64, :], avs[0:64, :], BCs[:])
                    else:
                        OTt = otmp.tile([64, 512], bf16, tag="otmp", name="OTt")
                        nc.vector.tensor_mul(OTt[:], avs[0:64, :], BCs[:])
                        nc.sync.dma_start(out=OT[64:128, :], in_=OTt[:])

                if L < 2 * QC - 1:
                    at(L + 1, 1, (lambda c=chain: c(0)))
                    at(L + 1, 3, (lambda c=chain: c(1)))
                else:
                    chain(0)
                    chain(1)

                # spread Wo(qc) pieces across the NEXT hp==1 ladder
                if hp == 1 and qc < QC - 1:
                    for o8 in range(8):
                        at(L + 2, 4 + o8,
                           (lambda q=qc, o=o8:
                            emit_wo_piece(q, [OTs[(q, 0)], OTs[(q, 1)]],
                                          yp, 'yp', o)))

            emit_wo(QC - 1, [OTs[(QC - 1, 0)], OTs[(QC - 1, 1)]],
                    [(yp, 'yp'), (avop, 'av0'), (avop, 'av1')])

    nc.compile()
    return nc


def _get_nc():
    global _NC
    if _NC is None:
        _NC = _build_nc()
    return _NC


def _host_fallback(query, keys, values, mask, Wq, Wk, Wv, Wo):
    # Exact reference math in numpy; only used if mask has zeros (off-spec).
    q = (query @ Wq.T).reshape(SL, BS, H, DH)
    k = (keys @ Wk.T).reshape(SL, BS, H, DH)
    v = (values @ Wv.T).reshape(SL, BS, H, DH)
    out = np.zeros((SL, BS, H * DH), np.float32)
    for b in range(BS):
        for h in range(H):
            s = q[:, b, h, :] @ k[:, b, h, :].T
            s = np.where(mask[0, 0] == 0, np.float32(-1e20), s)
            s = s - s.max(axis=-1, keepdims=True)
            p = np.exp(s)
            p /= p.sum(axis=-1, keepdims=True)
            out[:, b, h * DH:(h + 1) * DH] = p @ v[:, b, h, :]
    return out @ Wo.T


def _enable_trace_support():
    """Install the antenv.axon_hooks shim so trace=True works under axon."""
    import sys
    import types
    import antenv
    if "antenv.axon_hooks" in sys.modules:
        return
    hookmod = types.ModuleType("antenv.axon_hooks")
    _hook = [None]
    hookmod.set_axon_ntff_profile_hook = lambda h: _hook.__setitem__(0, h)
    hookmod.get_axon_ntff_profile_hook = lambda: _hook[0]
    antenv.axon_hooks = hookmod
    sys.modules["antenv.axon_hooks"] = hookmod
    try:
        from trn_agent_boot.trn_boot import _ntff_profile_via_ctypes
        hookmod.set_axon_ntff_profile_hook(
            _ntff_profile_via_ctypes("/opt/axon/libaxon_pjrt.so"))
    except Exception:
        pass
    import concourse.bass_utils as bu
    bu.upload_artifacts = lambda tmpdir: tmpdir


def _w_sb_layout(Wslice):
    # [256 od, 1024 D] -> [128 p, dc*256+od]
    return np.ascontiguousarray(
        Wslice.reshape(OD, DC, 128).transpose(2, 1, 0).reshape(128, DC * OD))


def _wo_sb_layout(WoSlice):
    # [1024 o, 256 hd] -> [128 p, hp*1024+o]
    return np.ascontiguousarray(
        WoSlice.reshape(D, 2, 128).transpose(2, 1, 0).reshape(128, 2 * D))


def kernel(query, keys, values, mask, Wq, Wk, Wv, Wo):
    query = np.asarray(query, np.float32)
    keys = np.asarray(keys, np.float32)
    values = np.asarray(values, np.float32)
    mask = np.asarray(mask)
    Wq = np.asarray(Wq, np.float32)
    Wk = np.asarray(Wk, np.float32)
    Wv = np.asarray(Wv, np.float32)
    Wo = np.asarray(Wo, np.float32)

    if (mask == 0).any():
        return _host_fallback(query, keys, values, mask, Wq, Wk, Wv, Wo)

    trace = bool(int(os.environ.get("KERNEL_TRACE", "0")))
    if trace:
        _enable_trace_support()

    from concourse.bass_utils import run_bass_kernel_spmd

    nc = _get_nc()
    in_maps = []
    for c in range(NCORES):
        b, hg = divmod(c, 4)
        hs = hg * OD
        in_maps.append({
            "xqT": np.ascontiguousarray(query[:, b, :].T),
            "xkT": np.ascontiguousarray(keys[:, b, :].T),
            "xvT": np.ascontiguousarray(values[:, b, :].T).astype(ml_dtypes.bfloat16),
            "wqT": _w_sb_layout(Wq[hs:hs + OD, :]),
            "wkT": _w_sb_layout(Wk[hs:hs + OD, :]),
            "wvT": _w_sb_layout(Wv[hs:hs + OD, :]).astype(ml_dtypes.bfloat16),
            "woT": _wo_sb_layout(Wo[:, hs:hs + OD]).astype(ml_dtypes.bfloat16),
            "onesd": np.ones((128, 260), np.float32),
            "onesvd": np.ones((128, 260), ml_dtypes.bfloat16),
        })

    res = run_bass_kernel_spmd(nc, in_maps, core_ids=list(range(NCORES)),
                               trace=trace)
    global LAST_RESULT
    LAST_RESULT = res

    out = np.zeros((SL, BS, D), np.float32)
    for c in range(NCORES):
        b = c // 4
        out[:, b, :] += res.results[c]["yT"].T
    return out


# revision 25
# speedup vs baseline: 1.3222x; 1.0649x over previous
"""Trainium2 Bass kernel for nn_MultiHeadAttention (SL=2048, BS=2, D=1024, H=16, DH=64).

Sharding: the [BS=2, H=16] grid of attention heads is split across 8 cores:
core c handles batch b = c//4 and heads 4*(c%4) .. 4*(c%4)+4.
Each core computes q/k/v projections for its own head slice, the 4 attention
maps, and a partial output (its heads' contribution through Wo). The host
sums the 4 partials per batch.

All matmuls run as float32r (fp32 storage, FP22 multiply) at full PE rate.
Scores are computed transposed (S^T[k, q]) so softmax-exp output feeds the
AV matmul directly; softmax denominators come from an ones-matmul
(column-sum over PSUM partitions), replicated across 64 partitions so the
normalization is a plain elementwise multiply.
"""

import os
import ml_dtypes
import numpy as np

SL, BS, D = 2048, 2, 1024
H, DH = 16, 64
NCORES = 8
HPC = 4            # heads per core
OD = HPC * DH      # 256 projected dims per core
DC = D // 128      # 8 contraction chunks
QC = SL // 512     # 4 query chunks of 512
KT = SL // 128     # 16 key tiles of 128

_NC = None
LAST_RESULT = None


def _build_nc():
    import concourse.mybir as mybir
    import concourse.tile as tile
    from concourse import bacc

    f32 = mybir.dt.float32
    f32r = mybir.dt.float32r
    bf16 = mybir.dt.bfloat16
    f16 = mybir.dt.float16
    EXP = mybir.ActivationFunctionType.Exp

    nc = bacc.Bacc(None, target_bir_lowering=False, debug=True)

    xqT = nc.dram_tensor("xqT", [D, SL], f16, kind="ExternalInput")
    xkT = nc.dram_tensor("xkT", [D, SL], f16, kind="ExternalInput")
    xvT = nc.dram_tensor("xvT", [D, SL], f16, kind="ExternalInput")
    wqT = nc.dram_tensor("wqT", [128, DC * OD], f16, kind="ExternalInput")
    wkT = nc.dram_tensor("wkT", [128, DC * OD], f16, kind="ExternalInput")
    wvT = nc.dram_tensor("wvT", [128, DC * OD], f16, kind="ExternalInput")
    woT = nc.dram_tensor("woT", [128, 2 * D], f16, kind="ExternalInput")
    onesd = nc.dram_tensor("onesd", [128, 260], f32r, kind="ExternalInput")
    onesvd = nc.dram_tensor("onesvd", [128, 260], bf16, kind="ExternalInput")
    yT = nc.dram_tensor("yT", [D, SL], f32, kind="ExternalOutput")
    debug = bool(int(os.environ.get("KERNEL_DEBUG", "0")))
    if debug:
        dbg_qT = [nc.dram_tensor(f"dbg_qT{i}", [128, SL], f32r, kind="ExternalOutput") for i in range(2)]
        dbg_kT = [nc.dram_tensor(f"dbg_kT{i}", [128, SL], f32r, kind="ExternalOutput") for i in range(2)]
        dbg_v0 = nc.dram_tensor("dbg_v0", [128, 260], bf16, kind="ExternalOutput")
        dbg_E = nc.dram_tensor("dbg_E", [128, 1024], bf16, kind="ExternalOutput")
        dbg_OT = nc.dram_tensor("dbg_OT", [128, 512], f32r, kind="ExternalOutput")
        dbg_AVO = [nc.dram_tensor(f"dbg_AVO{i}", [65, 512], f32, kind="ExternalOutput") for i in range(2)]
        dbg_BC = [nc.dram_tensor(f"dbg_BC{i}", [64, 512], f32, kind="ExternalOutput") for i in range(2)]
        dbg_rec = [nc.dram_tensor(f"dbg_rec{i}", [1, 512], f32, kind="ExternalOutput") for i in range(2)]
        dbg_recr = [nc.dram_tensor(f"dbg_recr{i}", [1, 512], f32r, kind="ExternalOutput") for i in range(2)]

    with tile.TileContext(nc) as tc:
        with (
            tc.tile_pool(name="wsb", bufs=1) as wsb,
            tc.tile_pool(name="qk", bufs=1) as qk,
            tc.tile_pool(name="vsb", bufs=1) as vsb,
            tc.tile_pool(name="xkp", bufs=8) as xkp,
            tc.tile_pool(name="xqp", bufs=8) as xqp,
            tc.tile_pool(name="xvp", bufs=16) as xvp,
            tc.tile_pool(name="esb", bufs=3) as esb,
            tc.tile_pool(name="rsb", bufs=2) as rsb,
            tc.tile_pool(name="otsb", bufs=4) as otsb,
            tc.tile_pool(name="ysb", bufs=3) as ysb,
            tc.tile_pool(name="otmp", bufs=2) as otmp,
            tc.tile_pool(name="avsb", bufs=2) as avsb,
            tc.tile_pool(name="pp", bufs=1, space="PSUM") as pp,
            tc.tile_pool(name="wp", bufs=2, space="PSUM") as wp,
            tc.tile_pool(name="avop", bufs=1, space="PSUM") as avop,
            tc.tile_pool(name="yp", bufs=1, space="PSUM") as yp,
        ):
            # --- persistent SBUF tensors ---
            wq_sb = wsb.tile([128, DC * OD], f16, tag="wq")  # [p, dc*256+od]
            wk_sb = wsb.tile([128, DC * OD], f16, tag="wk")
            wv_sb = wsb.tile([128, DC * OD], f16, tag="wv")
            wo_sb = wsb.tile([128, 2 * D], f16, tag="wo")    # [p, hp*1024+o]
            ones_sb = wsb.tile([128, 260], f32r, tag="ones")
            kT_sb = [qk.tile([128, SL], f32r, tag=f"kT{ot}", name=f"kT{ot}") for ot in range(2)]
            qT_sb = [qk.tile([128, SL], f32r, tag=f"qT{ot}", name=f"qT{ot}") for ot in range(2)]
            v_sb = [vsb.tile([128, 260], bf16, tag=f"v{t}", name=f"v{t}") for t in range(KT)]

            def load_w(dst, src):
                nc.sync.dma_start(out=dst[:], in_=src[:])

            def load_x(pool, tg, xdram, cc, dt_=f32r):
                tiles = []
                for d in range(DC):
                    t = pool.tile([128, 512], dt_, tag=tg, name="x")
                    nc.sync.dma_start(out=t[:],
                                      in_=xdram[d * 128:(d + 1) * 128,
                                                cc * 512:(cc + 1) * 512])
                    tiles.append(t[:])
                return tiles

            def load_x2(pool, tg, xdram, cc2, dt_=f32r):
                # one [128, 1024] DMA per D-chunk: bigger descriptors, twice
                # the per-queue bandwidth; returns slice lists for both
                # 512-token halves
                ev, od = [], []
                for d in range(DC):
                    t = pool.tile([128, 1024], dt_, tag=tg, name="x")
                    nc.sync.dma_start(out=t[:],
                                      in_=xdram[d * 128:(d + 1) * 128,
                                                cc2 * 1024:(cc2 + 1) * 1024])
                    ev.append(t[:, 0:512])
                    od.append(t[:, 512:1024])
                return ev, od

            qp_ps = {}

            def proj_qk_quarter(w_sb, dst, cc, xtiles, ot, half):
                if half == 0:
                    qp_ps[(cc, ot)] = pp.tile([128, 512], f32, tag="pp",
                                              name="ps")
                ps = qp_ps[(cc, ot)]
                for d in range(half * 4, half * 4 + 4):
                    nc.tensor.matmul(
                        ps[:],
                        (w_sb[:, d * OD + ot * 128: d * OD + (ot + 1) * 128]),
                        (xtiles[d][:]),
                        start=(d == 0), stop=(d == DC - 1))
                if half == 1:
                    nc.vector.tensor_copy(dst[ot][:, cc * 512:(cc + 1) * 512],
                                          ps[:])

            def proj_qk_half(w_sb, dst, cc, xtiles, ot):
                proj_qk_quarter(w_sb, dst, cc, xtiles, ot, 0)
                proj_qk_quarter(w_sb, dst, cc, xtiles, ot, 1)

            def proj_qk(w_sb, dst, cc, xtiles):
                for ot in range(2):
                    proj_qk_half(w_sb, dst, cc, xtiles, ot)

            def emit_qproj(qc_):
                xt = load_x(xqT, qc_)
                proj_qk(wq_sb, qT_sb, qc_, xt)

            def emit_wo_piece(qc_, ot_tiles, pool, ptag, o8):
                Y = pool.tile([128, 512], f32, tag=ptag, name="Y")
                for hp in range(2):
                    nc.tensor.matmul(
                        Y[:],
                        (wo_sb[:, hp * D + o8 * 128: hp * D + (o8 + 1) * 128]),
                        (ot_tiles[hp][:]),
                        start=(hp == 0), stop=(hp == 1))
                ys = ysb.tile([128, 512], f32, tag="ys", name="ys")
                nc.vector.tensor_copy(ys[:], Y[:])
                nc.sync.dma_start(
                    out=yT[o8 * 128:(o8 + 1) * 128, qc_ * 512:(qc_ + 1) * 512],
                    in_=ys[:])

            def emit_wo(qc_, ot_tiles, pools):
                for o8 in range(8):
                    pool, ptag = pools[o8 % len(pools)]
                    emit_wo_piece(qc_, ot_tiles, pool, ptag, o8)

            # --- projections: k fully, q chunk 0; v is produced inside the
            # first attention ladder, one 128-token tile per window ---
            nc.sync.dma_start(out=ones_sb[:], in_=onesd[:])
            load_w(wk_sb, wkT)
            warm = yp.tile([128, 512], f32, tag="yp", name="warm")
            for i in range(24):
                nc.tensor.matmul(warm[0:64, 0:256], ones_sb[:, 0:64],
                                 ones_sb[:, 0:256], start=(i == 0),
                                 stop=(i == 23))
            warms = ysb.tile([64, 256], f32, tag="ys", name="warms")
            nc.vector.tensor_copy(warms[:], warm[0:64, 0:256])
            xq0 = None
            for cc2 in range(2):
                ev, od = load_x2(xkp, 'xk', xkT, cc2, f16)
                if cc2 == 0:
                    xq0 = load_x(xqp, 'xq', xqT, 0, f16)
                    load_w(wq_sb, wqT)
                    load_w(wv_sb, wvT)
                proj_qk(wk_sb, kT_sb, 2 * cc2, ev)
                proj_qk(wk_sb, kT_sb, 2 * cc2 + 1, od)
            proj_qk(wq_sb, qT_sb, 0, xq0)
            nc.sync.dma_start(out=wo_sb[:], in_=woT[:])
            _xv01 = load_x2(xvp, 'xv', xvT, 0, f16)
            xv_chunks = {0: _xv01[0], 1: _xv01[1]}

            def emit_vtile(t_):
                cc_, tt = divmod(t_, 4)
                xtiles = xv_chunks[cc_]
                nc.sync.dma_start(out=v_sb[t_][:], in_=onesvd[:])
                ps = pp.tile([128, OD], f32, tag="pp", name="ps")
                for d in range(DC):
                    nc.tensor.matmul(
                        ps[:],
                        (xtiles[d][:, tt * 128:(tt + 1) * 128]),
                        (wv_sb[:, d * OD:(d + 1) * OD]),
                        start=(d == 0), stop=(d == DC - 1))
                for h in range(4):
                    nc.vector.tensor_copy(
                        v_sb[t_][:, h * 65:h * 65 + 64],
                        ps[:, h * 64:(h + 1) * 64])
            if debug:
                for i in range(2):
                    nc.sync.dma_start(out=dbg_kT[i][:], in_=kT_sb[i][:])
                nc.sync.dma_start(out=dbg_v0[:], in_=v_sb[0][:])

            # --- attention: 8 ladders (qc-major, head-pair minor), with
            # fine-grained insertions so ACT stays saturated ---
            inserts = {}

            def at(L_, kt_, fn):
                inserts.setdefault((L_, kt_), []).append(fn)

            OTs = {}

            # schedule q-projections (per-ot halves) and xq loads
            xq_t = {}
            for qc_ in range(1, QC):
                Lt = 1 if qc_ == 1 else (qc_ - 1) * 2
                at(Lt, 4, (lambda q=qc_: xq_t.__setitem__(q, load_x(xqp, 'xq', xqT, q, f16))))
                for j in range(4):
                    at(Lt, 5 + 2 * j,
                       (lambda q=qc_, ot=j // 2, hf=j % 2:
                        proj_qk_quarter(wq_sb, qT_sb, q, xq_t[q], ot, hf)))

            for L in range(2 * QC):
                qc, hp = divmod(L, 2)
                AVO = [avop.tile([65, 512], f32, tag=f"av{hip}", name="AVO")
                       for hip in range(2)]

                def emit_av(E_, kt_, AVO=AVO, hp=hp):
                    for hip in range(2):
                        nc.tensor.matmul(
                            AVO[hip][:],
                            (v_sb[kt_][:, (hp * 2 + hip) * 65:
                                         (hp * 2 + hip) * 65 + 65]),
                            (E_[:, hip * 512:(hip + 1) * 512]),
                            start=(kt_ == 0), stop=(kt_ == KT - 1))

                prev_E = None
                for kt in range(KT):
                    W = wp.tile([128, 1024], f32, tag="wp", name="W")
                    for hip in range(2):
                        nc.tensor.matmul(
                            W[:, hip * 512:(hip + 1) * 512],
                            (kT_sb[hp][hip * 64:(hip + 1) * 64,
                                         kt * 128:(kt + 1) * 128]),
                            (qT_sb[hp][hip * 64:(hip + 1) * 64,
                                         qc * 512:(qc + 1) * 512]),
                            start=True, stop=True)
                    E = esb.tile([128, 1024], bf16, tag="E", name="E")
                    nc.scalar.activation(E[:], W[:], EXP)
                    if debug and L == 0 and kt == 0:
                        nc.sync.dma_start(out=dbg_E[:], in_=E[:])
                        for i in range(2):
                            nc.sync.dma_start(out=dbg_qT[i][:, 0:512],
                                              in_=qT_sb[i][:, 0:512])
                    if prev_E is not None:
                        emit_av(prev_E, kt - 1)
                    prev_E = E
                    if L == 0:
                        emit_vtile(kt)
                        if kt == 1:
                            _xv23 = load_x2(xvp, 'xv', xvT, 1, f16)
                            xv_chunks[2] = _xv23[0]
                            xv_chunks[3] = _xv23[1]
                    for fn in inserts.pop((L, kt), []):
                        fn()
                emit_av(prev_E, KT - 1)

                # evacuate accumulators promptly, then defer the normalize
                # chain into the next ladder
                avs_pair = []
                for hip in range(2):
                    avs = avsb.tile([65, 512], f32, tag="avs", name="avs")
                    nc.vector.tensor_copy(avs[:], AVO[hip][:])
                    avs_pair.append(avs)
                OT = otsb.tile([128, 512], f16, tag="ot", name="OT")
                OTs[(qc, hp)] = OT

                def chain(hip, avs_pair=avs_pair, OT=OT):
                    avs = avs_pair[hip]
                    sums_r = rsb.tile([65, 512], f32r, tag="recip", name="sums_r")
                    nc.vector.tensor_copy(sums_r[64:65, :], avs[64:65, :])
                    BCp = pp.tile([64, 512], f32, tag="pp", name="BCp")
                    nc.tensor.matmul(BCp[:], ones_sb[64:65, 0:64],
                                     sums_r[64:65, :], start=True, stop=True)
                    sumsb = ysb.tile([64, 512], f32, tag="ys", name="sumsb")
                    nc.vector.tensor_copy(sumsb[:], BCp[:])
                    BCs = ysb.tile([64, 512], f32, tag="ys", name="BCs")
                    nc.vector.reciprocal_approx_fast(BCs[:], sumsb[:])
                    if hip == 0:
                        nc.vector.tensor_mul(OT[0:64, :], avs[0:64, :], BCs[:])
                    else:
                        OTt = otmp.tile([64, 512], f16, tag="otmp", name="OTt")
                        nc.vector.tensor_mul(OTt[:], avs[0:64, :], BCs[:])
                        nc.sync.dma_start(out=OT[64:128, :], in_=OTt[:])

                if L < 2 * QC - 1:
                    at(L + 1, 1, (lambda c=chain: c(0)))
                    at(L + 1, 3, (lambda c=chain: c(1)))
                else:
                    chain(0)
                    chain(1)

                # spread Wo(qc) pieces across the NEXT hp==1 ladder
                if hp == 1 and qc < QC - 1:
                    for o8 in range(8):
                        at(L + 2, 4 + o8,
                           (lambda q=qc, o=o8:
                            emit_wo_piece(q, [OTs[(q, 0)], OTs[(q, 1)]],
                                          yp, 'yp', o)))

            emit_wo(QC - 1, [OTs[(QC - 1, 0)], OTs[(QC - 1, 1)]],
                    [(yp, 'yp'), (avop, 'av0'), (avop, 'av1')])

    nc.compile()
    return nc


def _get_nc():
    global _NC
    if _NC is None:
        _NC = _build_nc()
    return _NC


def _host_fallback(query, keys, values, mask, Wq, Wk, Wv, Wo):
    # Exact reference math in numpy; only used if mask has zeros (off-spec).
    q = (query @ Wq.T).reshape(SL, BS, H, DH)
    k = (keys @ Wk.T).reshape(SL, BS, H, DH)
    v = (values @ Wv.T).reshape(SL, BS, H, DH)
    out = np.zeros((SL, BS, H * DH), np.float32)
    for b in range(BS):
        for h in range(H):
            s = q[:, b, h, :] @ k[:, b, h, :].T
            s = np.where(mask[0, 0] == 0, np.float32(-1e20), s)
            s = s - s.max(axis=-1, keepdims=True)
            p = np.exp(s)
            p /= p.sum(axis=-1, keepdims=True)
            out[:, b, h * DH:(h + 1) * DH] = p @ v[:, b, h, :]
    return out @ Wo.T


def _enable_trace_support():
    """Install the antenv.axon_hooks shim so trace=True works under axon."""
    import sys
    import types
    import antenv
    if "antenv.axon_hooks" in sys.modules:
        return
    hookmod = types.ModuleType("antenv.axon_hooks")
    _hook = [None]
    hookmod.set_axon_ntff_profile_hook = lambda h: _hook.__setitem__(0, h)
    hookmod.get_axon_ntff_profile_hook = lambda: _hook[0]
    antenv.axon_hooks = hookmod
    sys.modules["antenv.axon_hooks"] = hookmod
    try:
        from trn_agent_boot.trn_boot import _ntff_profile_via_ctypes
        hookmod.set_axon_ntff_profile_hook(
            _ntff_profile_via_ctypes("/opt/axon/libaxon_pjrt.so"))
    except Exception:
        pass
    import concourse.bass_utils as bu
    bu.upload_artifacts = lambda tmpdir: tmpdir


def _w_sb_layout(Wslice):
    # [256 od, 1024 D] -> [128 p, dc*256+od]
    return np.ascontiguousarray(
        Wslice.reshape(OD, DC, 128).transpose(2, 1, 0).reshape(128, DC * OD))


def _wo_sb_layout(WoSlice):
    # [1024 o, 256 hd] -> [128 p, hp*1024+o]
    return np.ascontiguousarray(
        WoSlice.reshape(D, 2, 128).transpose(2, 1, 0).reshape(128, 2 * D))


def kernel(query, keys, values, mask, Wq, Wk, Wv, Wo):
    query = np.asarray(query, np.float32)
    keys = np.asarray(keys, np.float32)
    values = np.asarray(values, np.float32)
    mask = np.asarray(mask)
    Wq = np.asarray(Wq, np.float32)
    Wk = np.asarray(Wk, np.float32)
    Wv = np.asarray(Wv, np.float32)
    Wo = np.asarray(Wo, np.float32)

    if (mask == 0).any():
        return _host_fallback(query, keys, values, mask, Wq, Wk, Wv, Wo)

    trace = bool(int(os.environ.get("KERNEL_TRACE", "0")))
    if trace:
        _enable_trace_support()

    from concourse.bass_utils import run_bass_kernel_spmd

    nc = _get_nc()
    in_maps = []
    for c in range(NCORES):
        b, hg = divmod(c, 4)
        hs = hg * OD
        in_maps.append({
            "xqT": np.ascontiguousarray(query[:, b, :].T).astype(np.float16),
            "xkT": np.ascontiguousarray(keys[:, b, :].T).astype(np.float16),
            "xvT": np.ascontiguousarray(values[:, b, :].T).astype(np.float16),
            "wqT": _w_sb_layout(Wq[hs:hs + OD, :]).astype(np.float16),
            "wkT": _w_sb_layout(Wk[hs:hs + OD, :]).astype(np.float16),
            "wvT": _w_sb_layout(Wv[hs:hs + OD, :]).astype(np.float16),
            "woT": _wo_sb_layout(Wo[:, hs:hs + OD]).astype(np.float16),
            "onesd": np.ones((128, 260), np.float32),
            "onesvd": np.ones((128, 260), ml_dtypes.bfloat16),
        })

    res = run_bass_kernel_spmd(nc, in_maps, core_ids=list(range(NCORES)),
                               trace=trace)
    global LAST_RESULT
    LAST_RESULT = res

    out = np.zeros((SL, BS, D), np.float32)
    for c in range(NCORES):
        b = c // 4
        out[:, b, :] += res.results[c]["yT"].T
    return out
